# revision 15
# baseline (speedup 1.0000x reference)
"""GAT (2-layer graph attention) on 8 Trainium2 NeuronCores.

Node tables (256B rows) are AllGather'd and per-edge rows fetched with
dma_gather. int16 gather indices cap tables at 32768 rows; the rank space
is QUARTER-MAJOR (local node r -> quarter r // QREAL), so each of the 4
windows equals one quarter of every core's shard and each layer's AllGather
splits into 4 quarter-AGs launched as their producer chunk (25 dense blocks
/ 1 combine chunk) finishes — collectives pipeline with dense/combine and
with the edge-phase windows, and window stats balance (G_w uniform, less
K-padding). Each (dst-node, window) pair is a degree-sorted "virtual row"
producing partial softmax sums, combined by a second gather round. Softmax
max-subtraction is dropped (shift-invariant; logits are O(1)).

Row layout v2: layer-1 head h occupies [9h:9h+9) = 8 features + a baked
1.0 column (layer 2: h2[0:40] | 1.0@40 | asrc@41), so the attention-softmax
denominator falls out of the same bf16 msg-multiply + k-tree-reduce as the
weighted message — no separate strided reduce, no f32 packing, no memsets.
"""

import os as _os

import numpy as np
import ml_dtypes

bf16 = ml_dtypes.bfloat16

# ---------------- problem constants -----------------------------------
N = 100000
E = 1600000
NC = 8
F_IN = 512
H1, D1 = 8, 8
HD1 = H1 * D1
C = 40
NEG_SLOPE = 0.2
EPS = 1e-16

REAL = N // NC
BLOCKS = 100
SHARD = BLOCKS * 128
RANKS = NC * SHARD
# Quarter-major rank space: local node r lives in quarter q = r // QREAL at
# padded row q*QROWS + r % QREAL; window w == quarter w across all cores, so
# each layer's AllGather splits into 4 independent quarter-AGs that pipeline
# with the dense/combine producers and the edge-phase consumers.
QN = 4
QROWS = SHARD // QN          # 3200 padded rows per (core, quarter)
QREAL = REAL // QN           # 3125 real rows per (core, quarter)
WIN = NC * QROWS             # 25600 <= 32767 (int16 gather indices)
NWIN = QN
RW = 128                 # bf16 elems per table row (256B)
BATCH_KMAX = int(_os.environ.get("KBM", "64"))  # max sum-of-K per gather call
CHUNK = 25               # combine blocks per chunk
ADST_GMAX = int(_os.environ.get("KADG", "64"))


def _wrap_idx(flat):
    n = flat.shape[0]
    assert n % 16 == 0
    w16 = flat.reshape(n // 16, 16).T
    return np.tile(w16, (8, 1)).astype(np.int16)


# Pad slots must NOT all hit one ghost row: 17% of descriptors hammering a
# single 256B HBM row serializes one bank and slows the whole gather stream
# ~2.8x (5.3 vs 1.9 ns/desc measured). Spread over all NC*(QROWS-QREAL)
# ghost rows.
GHOST_ROWS = np.concatenate(
    [c * QROWS + QREAL + np.arange(QROWS - QREAL) for c in range(NC)])


def preprocess(edge_index):
    src = np.asarray(edge_index[0], np.int64)
    dst = np.asarray(edge_index[1], np.int64)
    loops = np.arange(N, dtype=np.int64)
    src = np.concatenate([src, loops])
    dst = np.concatenate([dst, loops])

    core = dst // REAL
    rl = dst % REAL
    r_local = (rl // QREAL) * QROWS + rl % QREAL      # padded dst row
    src_c = src // REAL
    src_r = src % REAL
    w = src_r // QREAL                                 # window == quarter
    rel = src_c * QROWS + src_r % QREAL                # rank within window

    key = (core * NWIN + w) * SHARD + r_local
    deg = np.bincount(key, minlength=NC * NWIN * SHARD).reshape(
        NC, NWIN, SHARD)

    vlists = {}
    nnz = np.zeros((NC, NWIN), int)
    for c in range(NC):
        for wi in range(NWIN):
            d = deg[c, wi]
            rs = np.nonzero(d)[0]
            order = np.argsort(-d[rs], kind="stable")
            rs = rs[order]
            vlists[(c, wi)] = (rs, d[rs])
            nnz[c, wi] = len(rs)

    G_w = [max(1, int(np.ceil(nnz[:, wi].max() / 128))) for wi in range(NWIN)]
    K_w = []
    for wi in range(NWIN):
        ks = np.zeros(G_w[wi], int)
        for c in range(NC):
            degs = vlists[(c, wi)][1]
            for g in range(G_w[wi]):
                if g * 128 < len(degs):
                    ks[g] = max(ks[g], degs[g * 128])
        ks = np.maximum(ks, 1)
        K_w.append(ks)

    # K-uniform batches: every group in a batch is padded to the batch max K
    # so per-group DVE ops fuse into single per-batch ops.
    batches_w = []
    for wi in range(NWIN):
        batches = []
        g0 = 0
        while g0 < G_w[wi]:
            kb = int(K_w[wi][g0])          # groups sorted by K desc
            nb = max(1, min(BATCH_KMAX // kb, G_w[wi] - g0))
            g1 = g0 + nb
            K_w[wi][g0:g1] = kb            # pad K uniform within batch
            batches.append((g0, g1, int(kb * nb)))
            g0 = g1
        batches_w.append(batches)

    cumK_w = [np.concatenate([[0], np.cumsum(K_w[wi])]) for wi in range(NWIN)]

    sortpos = np.full((NC, NWIN, SHARD), -1, np.int64)
    for c in range(NC):
        for wi in range(NWIN):
            rs = vlists[(c, wi)][0]
            sortpos[c, wi, rs] = np.arange(len(rs))
    vpos = sortpos[core, w, r_local]
    order = np.argsort(key, kind="stable")
    k_in_row = np.empty(len(key), np.int64)
    sk_ = key[order]
    first = np.concatenate([[True], sk_[1:] != sk_[:-1]])
    starts = np.nonzero(first)[0]
    run_id = np.cumsum(first) - 1
    k_in_row[order] = np.arange(len(key)) - starts[run_id]

    gv = vpos // 128
    pv = vpos % 128

    idx_e_cores, idx_a_cores, idx_c_cores = [], [], []
    for c in range(NC):
        e_parts = []
        m_c = core == c
        for wi in range(NWIN):
            ncol = int(cumK_w[wi][-1])
            A = GHOST_ROWS[np.arange(128 * ncol) % len(GHOST_ROWS)].reshape(
                128, ncol)
            m = m_c & (w == wi)
            col = cumK_w[wi][gv[m]] + k_in_row[m]
            A[pv[m], col] = rel[m]
            for (g0, g1, _sk) in batches_w[wi]:
                c0, c1 = int(cumK_w[wi][g0]), int(cumK_w[wi][g1])
                e_parts.append(_wrap_idx(A[:, c0:c1].T.reshape(-1)))
        idx_e_cores.append(np.concatenate(e_parts, axis=1))

        a_parts = []
        for wi in range(NWIN):
            rs = vlists[(c, wi)][0]
            R_flat = np.zeros(G_w[wi] * 128, np.int64)
            R_flat[: len(rs)] = rs
            R = R_flat.reshape(G_w[wi], 128).T
            g0 = 0
            while g0 < G_w[wi]:
                g1 = min(g0 + ADST_GMAX, G_w[wi])
                a_parts.append(_wrap_idx(R[:, g0:g1].T.reshape(-1)))
                g0 = g1
        idx_a_cores.append(np.concatenate(a_parts, axis=1))

        c_parts = []
        for wi in range(NWIN):
            zr = 128 * G_w[wi]
            # spread no-virtual-row pads over 128 zero rows (same bank-
            # hammering fix as GHOST_ROWS)
            Cidx = zr + np.arange(SHARD, dtype=np.int64) % 128
            rs = vlists[(c, wi)][0]
            vp = np.arange(len(rs))
            Cidx[rs] = (vp % 128) * G_w[wi] + (vp // 128)
            Cm = Cidx.reshape(BLOCKS, 128).T
            for b0 in range(0, BLOCKS, CHUNK):
                b1 = min(b0 + CHUNK, BLOCKS)
                c_parts.append(_wrap_idx(Cm[:, b0:b1].T.reshape(-1)))
        idx_c_cores.append(np.concatenate(c_parts, axis=1))

    struct = dict(
        G_w=G_w, K_w=K_w, batches_w=batches_w, cumK_w=cumK_w,
        idx_e_w=idx_e_cores[0].shape[1], idx_a_w=idx_a_cores[0].shape[1],
        idx_c_w=idx_c_cores[0].shape[1],
    )
    return struct, idx_e_cores, idx_a_cores, idx_c_cores


# -----------------------------------------------------------------------
def build(struct):
    import os
    STAGE = int(os.environ.get("KSTAGE", "5"))
    KEDGE = int(os.environ.get("KEDGE", "3"))
    KREPS = int(os.environ.get("KREPS", "1"))
    KAG = int(os.environ.get("KAG", "1"))          # 0: fake AG with local copy
    # 2: skip AG entirely (timing-only; tables hold garbage)
    KADST = int(os.environ.get("KADST", "1"))      # 0: skip adst gathers
    # 0: Local AG outputs (fastest with pipelined quarter-AGs; >4 Shared
    # collectives also crash the runtime), 1: all Shared, 2: layer-1 Shared
    KSHARED = int(os.environ.get("KSHARED", "0"))
    KSP = bool(int(os.environ.get("KSP", "0")))    # single_packet for gathers
    # 1: leaky-relu on ACT engine (removes 2 DVE ops/batch, ~290 us; HW
    # Lrelu numerics are looser — rel err 1.4e-3 vs 4.8e-5, gate is 2e-2)
    KLR = int(os.environ.get("KLR", "1"))
    # 1: use Prelu instead of Lrelu — same math (x>0 ? x : alpha*x) but
    # Prelu shares the act-table set with Exp, killing ~126 table reloads
    KPRELU = int(os.environ.get("KPRELU", "1"))
    KEB = int(os.environ.get("KEB", "0"))          # 1: bf16 edge logits
    KCB = int(os.environ.get("KCB", "1"))          # deeper combine-gather
    # staging (cw tiles own pool): prefetch next chunk, -375 us
    KBUF = int(os.environ.get("KBUF", "0"))        # 1: trade a gather buf
    # for deeper compute pools (gp 3, ep/epb 3, epm 2); 0 wins now that
    # gathers are fast (ghost-spread fix): 3.60 vs 3.85 ms
    KGB = int(os.environ.get("KGB", "0"))          # override gather bufs
    import concourse.bacc as bacc
    import concourse.mybir as mybir
    import concourse.tile as tile
    from concourse.masks import make_identity

    F32 = mybir.dt.float32
    BF = mybir.dt.bfloat16
    I16 = mybir.dt.int16
    AX = mybir.AxisListType.X
    OP = mybir.AluOpType
    ACT = mybir.ActivationFunctionType

    G_w, K_w, batches_w = struct["G_w"], struct["K_w"], struct["batches_w"]
    cumK_w = struct["cumK_w"]
    KMAX = int(max(max(k) for k in K_w))
    FC = F_IN // 128

    nc = bacc.Bacc("TRN2", target_bir_lowering=False, debug=False,
                   num_devices=NC, num_swdge_queues=4)

    xt = nc.dram_tensor("xt", [F_IN, SHARD], F32, kind="ExternalInput").ap()
    w1 = nc.dram_tensor("w1", [F_IN, HD1], F32, kind="ExternalInput").ap()
    w2 = nc.dram_tensor("w2", [HD1, C], F32, kind="ExternalInput").ap()
    vec_in = {}
    for nm, width in [("atts1", HD1), ("attd1", HD1), ("b1", HD1),
                      ("atts2", C), ("attd2", C), ("b2", C)]:
        vec_in[nm] = nc.dram_tensor(nm, [1, width], F32,
                                    kind="ExternalInput").ap()
    idx_e = nc.dram_tensor("idx_e", [128, struct["idx_e_w"]], I16,
                           kind="ExternalInput").ap()
    idx_a = nc.dram_tensor("idx_a", [128, struct["idx_a_w"]], I16,
                           kind="ExternalInput").ap()
    idx_c = nc.dram_tensor("idx_c", [128, struct["idx_c_w"]], I16,
                           kind="ExternalInput").ap()
    out = nc.dram_tensor("out", [SHARD, C], F32, kind="ExternalOutput").ap()

    rg = [list(range(NC))]
    PT_rows = [128 * G_w[wi] + 128 for wi in range(NWIN)]
    PT_total = sum(PT_rows)
    PT_base = np.concatenate([[0], np.cumsum(PT_rows)]).astype(int)

    with tile.TileContext(nc) as tc:
        for _rep in range(KREPS):
            with (
                tc.tile_pool(name="dram", bufs=1, space="DRAM") as dpool,
                tc.tile_pool(name="setup", bufs=1) as sup,
                tc.tile_pool(name="psum0", bufs=2, space="PSUM") as psp,
            ):
                Rshard1 = dpool.tile([SHARD, RW], BF, tag="rs1")
                Rshard2 = dpool.tile([SHARD, RW], BF, tag="rs2")
                # KSHARED: 0 = all Local, 1 = all Shared, 2 = layer-1 only
                asp1 = "Shared" if KSHARED in (1, 2) else "Local"
                asp2 = "Shared" if KSHARED == 1 else "Local"
                Rw1 = [dpool.tile([WIN, RW], BF, tag=f"rf1{q}",
                                  addr_space=asp1, name=f"Rw1_{q}")
                       for q in range(QN)]
                Rw2 = [dpool.tile([WIN, RW], BF, tag=f"rf2{q}",
                                  addr_space=asp2, name=f"Rw2_{q}")
                       for q in range(QN)]
                AdstT = dpool.tile([SHARD, RW], BF, tag="adt")
                Ptab1 = dpool.tile([PT_total, RW], BF, tag="pt1")
                Ptab2 = dpool.tile([PT_total, RW], BF, tag="pt2")

                ident = sup.tile([128, 128], F32)
                make_identity(nc, ident[:])
                ones_row = sup.tile([1, 128], F32)
                nc.vector.memset(ones_row[:], 1.0)

                w1_t = sup.tile([128, FC * HD1], F32)
                nc.sync.dma_start(
                    w1_t[:].rearrange("p (c n) -> p c n", c=FC),
                    w1.rearrange("(c p) n -> p c n", p=128),
                )
                w2_t = sup.tile([128, C], F32)
                nc.sync.dma_start(w2_t[0:HD1, :], w2[:, :])
                nc.sync.dma_start(w2_t[HD1:2 * HD1, :], w2[:, :])

                reps = {}
                for nm in ["atts1", "attd1", "b1", "atts2", "attd2", "b2"]:
                    width = HD1 if nm in ("atts1", "attd1", "b1") else C
                    v = sup.tile([1, width], F32, tag=f"v_{nm}")
                    nc.sync.dma_start(v[:], vec_in[nm][:, :])
                    ps = psp.tile([128, width], F32, tag="rep_ps")
                    nc.tensor.matmul(out=ps[:], lhsT=ones_row[:], rhs=v[:],
                                     start=True, stop=True)
                    r_ = sup.tile([128, width], F32, tag=f"rep_{nm}")
                    nc.vector.tensor_copy(r_[:], ps[:])
                    reps[nm] = r_

                ghost1 = sup.tile([128, 88], BF)
                nc.vector.memset(ghost1[:], 0.0)
                nc.vector.memset(ghost1[:, 72:80], -100.0)
                zrow = sup.tile([128, RW], BF)
                nc.vector.memset(zrow[:], 0.0)
                for wi in range(NWIN):
                    zr = int(PT_base[wi]) + 128 * G_w[wi]
                    nc.sync.dma_start(Ptab1[:][zr:zr + 128, :], zrow[:])
                    nc.sync.dma_start(Ptab2[:][zr:zr + 128, :], zrow[:])

                # ---------------- dense layer 1 ----------------
                with (
                    tc.tile_pool(name="d1", bufs=3) as dp,
                    tc.tile_pool(name="d1p", bufs=2, space="PSUM") as dpp,
                ):
                    for t in range(BLOCKS):
                        xtile = dp.tile([128, FC * 128], F32, tag="x")
                        nc.sync.dma_start(
                            xtile[:].rearrange("p (c n) -> p c n", c=FC),
                            xt.rearrange("(c p) n -> p c n", p=128)[
                                :, :, t * 128:(t + 1) * 128],
                        )
                        hps = dpp.tile([128, HD1], F32, tag="h")
                        for cc in range(FC):
                            nc.tensor.matmul(
                                out=hps[:],
                                lhsT=xtile[:].rearrange(
                                    "p (c n) -> p c n", c=FC)[:, cc, :],
                                rhs=w1_t[:].rearrange(
                                    "p (c n) -> p c n", c=FC)[:, cc, :],
                                start=(cc == 0), stop=(cc == FC - 1),
                            )
                        # row layout v2: head h at [9h:9h+8], 1.0 at 9h+8
                        # (embedded softmax-denominator column), asrc@72,
                        # adst@80; [88:] never read.
                        row = dp.tile([128, RW], BF, tag="row")
                        rowh = row[:][:, 0:72].rearrange(
                            "p (h x) -> p h x", x=9)
                        nc.vector.tensor_copy(
                            rowh[:, :, 0:8],
                            hps[:].rearrange("p (h d) -> p h d", h=H1))
                        nc.vector.memset(rowh[:, :, 8:9], 1.0)
                        asrc_f = dp.tile([128, H1], F32, tag="asrcf")
                        adst_f = dp.tile([128, H1], F32, tag="adstf")
                        tmp = dp.tile([128, HD1], F32, tag="tmp")
                        for nm, dst_ap in (("atts1", asrc_f), ("attd1", adst_f)):
                            nc.vector.tensor_tensor(
                                out=tmp[:], in0=hps[:], in1=reps[nm][:],
                                op=OP.mult)
                            nc.vector.tensor_reduce(
                                out=dst_ap[:],
                                in_=tmp[:].rearrange("p (h d) -> p h d", h=H1),
                                axis=AX, op=OP.add)
                        nc.vector.tensor_copy(row[:, 72:80], asrc_f[:])
                        nc.vector.tensor_copy(row[:, 80:88], adst_f[:])
                        nc.sync.dma_start(Rshard1[t * 128:(t + 1) * 128, :],
                                          row[:])
                        arow = dp.tile([128, 9], BF, tag="arow")
                        nc.vector.tensor_copy(arow[:, 0:8], adst_f[:])
                        nc.vector.memset(arow[:, 8:9], 0.0)
                        nc.sync.dma_start(AdstT[t * 128:(t + 1) * 128, 0:9],
                                          arow[:])

                        # quarter q done after its 25 blocks: write its ghost
                        # rows and launch its AllGather so transfer overlaps
                        # the remaining dense blocks and later edge windows
                        if (t + 1) % (QROWS // 128) == 0:
                            q = t // (QROWS // 128)
                            g0r = q * QROWS + QREAL
                            nc.sync.dma_start(
                                Rshard1[g0r:(q + 1) * QROWS, 0:88],
                                ghost1[: QROWS - QREAL, :])
                            if KAG == 1:
                                nc.gpsimd.collective_compute(
                                    "AllGather", OP.bypass, replica_groups=rg,
                                    ins=[Rshard1[:][q * QROWS:
                                                    (q + 1) * QROWS, :].opt()],
                                    outs=[Rw1[q].opt()])
                            elif KAG == 2:
                                nc.sync.dma_start(
                                    Rw1[q][:][0:128, :],
                                    Rshard1[q * QROWS:q * QROWS + 128, :])
                            else:
                                for c in range(NC):
                                    nc.sync.dma_start(
                                        Rw1[q][:][c * QROWS:
                                                  (c + 1) * QROWS, :],
                                        Rshard1[q * QROWS:(q + 1) * QROWS, :])

                # ---------------- edge phase ----------------
                qn = [0, None]

                def edge_phase(Rws, Ptab, layer):
                    e_col = 0
                    a_col = 0
                    # v2 rows: layer 1 packs head h at [9h:9h+9] (8 feats +
                    # const 1.0 denominator column), asrc@72, adst@0 in AdstT;
                    # layer 2: h2[0:40], 1.0@40, asrc@41, adst@8 in AdstT.
                    if layer == 1:
                        Hh, Xw, alo, dlo = H1, D1 + 1, 72, 0
                    else:
                        Hh, Xw, alo, dlo = 1, C + 1, 41, 8
                    width = Hh * Xw
                    for wi in range(NWIN):
                        Rwin = Rws[wi]
                        Gn = G_w[wi]
                        ecols_w = 8 * sum(sk for (_g0, _g1, sk) in batches_w[wi])
                        acols_w = Gn * 8
                        with (
                            tc.tile_pool(name=f"ad{layer}{wi}", bufs=1) as apool,
                            tc.tile_pool(name=f"eg{layer}{wi}",
                                         bufs=KGB or (4 - KBUF)) as gp,
                            tc.tile_pool(name=f"ep{layer}{wi}",
                                         bufs=2 + KBUF) as ep,
                            tc.tile_pool(name=f"em{layer}{wi}",
                                         bufs=1 + KBUF) as epm,
                            tc.tile_pool(name=f"eb{layer}{wi}",
                                         bufs=2 + KBUF) as epb,
                            tc.tile_pool(name=f"ix{layer}{wi}", bufs=1) as ixp,
                        ):
                            ixw = ixp.tile([128, ecols_w], I16, tag="ixw")
                            nc.sync.dma_start(ixw[:], idx_e[:, e_col: e_col + ecols_w])
                            ixaw = ixp.tile([128, acols_w], I16, tag="ixaw")
                            nc.sync.dma_start(ixaw[:], idx_a[:, a_col: a_col + acols_w])
                            ecol_loc = 0
                            acol_loc = 0
                            adstG = apool.tile([128, Gn * RW], BF, tag="adstG")
                            adstG_v = adstG[:].rearrange("p (g e) -> p g e", e=RW)
                            if not KADST:
                                nc.vector.memset(adstG[:], 0.0)
                            g0 = 0
                            while g0 < Gn:
                                g1 = min(g0 + ADST_GMAX, Gn)
                                nidx = (g1 - g0) * 128
                                if KADST:
                                    _gi = nc.gpsimd.dma_gather(
                                        out_ap=adstG_v[:, g0:g1, :],
                                        in_ap=AdstT[:, :],
                                        idxs_ap=ixaw[:, acol_loc: acol_loc + nidx // 16],
                                        num_idxs=nidx, num_idxs_reg=nidx,
                                        elem_size=RW, single_packet=KSP,
                                        queue_num=qn[0] % 4)
                                    if qn[1] is not None:
                                        tile.add_dep_helper(_gi.ins, qn[1].ins, sync=False,
                                                            reason="swdge order")
                                    qn[1] = _gi
                                    qn[0] += 1
                                a_col += nidx // 16
                                acol_loc += nidx // 16
                                g0 = g1

                            for bidx, (g0, g1, sk) in enumerate(batches_w[wi]):
                                Kb = int(K_w[wi][g0])
                                ng = g1 - g0
                                nidx = 128 * sk
                                ixe = ixw[:, ecol_loc: ecol_loc + nidx // 16]
                                e_col += nidx // 16
                                ecol_loc += nidx // 16
                                G = gp.tile([128, BATCH_KMAX * RW], BF, tag="G")
                                Gv = G[:].rearrange("p (k e) -> p k e", e=RW)
                                Gg = G[:][:, 0:sk * RW].rearrange(
                                    "p (g k e) -> p g k e", g=ng, k=Kb)
                                _gi = nc.gpsimd.dma_gather(
                                    out_ap=Gv[:, 0:sk, :],
                                    in_ap=Rwin[:, :],
                                    idxs_ap=ixe,
                                    num_idxs=nidx, num_idxs_reg=nidx,
                                    elem_size=RW, single_packet=KSP,
                                    queue_num=qn[0] % 4)
                                if qn[1] is not None:
                                    tile.add_dep_helper(_gi.ins, qn[1].ins, sync=False,
                                                        reason="swdge order")
                                qn[1] = _gi
                                qn[0] += 1

                                if KEDGE < 2:
                                    continue
                                EDT = BF if KEB else F32
                                eT = ep.tile([128, BATCH_KMAX * Hh], EDT,
                                             tag="eT")
                                eV = eT[:][:, 0:sk * Hh].rearrange(
                                    "p (g k h) -> p g k h", g=ng, k=Kb)
                                pT = ep.tile([128, BATCH_KMAX * Hh], BF, tag="pT")
                                pb = epb.tile([128, BATCH_KMAX * RW], BF, tag="pb")
                                pbV = pb[:].rearrange("p (g e) -> p g e", e=RW)

                                # e = a_src[src] + a_dst (one op per batch)
                                nc.vector.tensor_tensor(
                                    out=eV[:, :, :, :],
                                    in0=Gg[:, :, :, alo:alo + Hh],
                                    in1=adstG_v[:, g0:g1, dlo:dlo + Hh]
                                        .unsqueeze(2)
                                        .to_broadcast([128, ng, Kb, Hh]),
                                    op=OP.add)
                                # leaky relu + exp
                                if KLR:
                                    eL = ep.tile([128, BATCH_KMAX * Hh], EDT,
                                                 tag="eL")
                                    nc.scalar.activation(
                                        eL[:, : sk * Hh], eT[:, : sk * Hh],
                                        ACT.Prelu if KPRELU else ACT.Lrelu,
                                        alpha=NEG_SLOPE)
                                    nc.scalar.activation(
                                        pT[:, : sk * Hh], eL[:, : sk * Hh],
                                        ACT.Exp)
                                else:
                                    ee = ep.tile([128, BATCH_KMAX * Hh], EDT,
                                                 tag="ee")
                                    nc.vector.tensor_scalar_mul(
                                        ee[:, : sk * Hh], eT[:, : sk * Hh],
                                        NEG_SLOPE)
                                    nc.vector.tensor_tensor(
                                        out=eT[:, : sk * Hh],
                                        in0=eT[:, : sk * Hh],
                                        in1=ee[:, : sk * Hh], op=OP.max)
                                    nc.scalar.activation(
                                        pT[:, : sk * Hh], eT[:, : sk * Hh],
                                        ACT.Exp)

                                # msg = p * [h | 1] (bf16; denominator rides in
                                # the const-1 column, so no separate reduce)
                                msg = epm.tile([128, BATCH_KMAX * width], BF,
                                               tag="msg")
                                msgV = msg[:][:, 0:sk * width].rearrange(
                                    "p (g k f) -> p g k f", g=ng, k=Kb)
                                nc.vector.tensor_tensor(
                                    out=msg[:][:, 0:sk * width].rearrange(
                                        "p (k h x) -> p k h x", k=sk, h=Hh),
                                    in0=Gv[:, 0:sk, 0:width].rearrange(
                                        "p k (h x) -> p k h x", h=Hh),
                                    in1=pT[:][:, 0:sk * Hh].rearrange(
                                        "p (k h) -> p k h", h=Hh)
                                        .unsqueeze(3)
                                        .to_broadcast([128, sk, Hh, Xw]),
                                    op=OP.mult)
                                # tree-reduce over k (uniform Kb, bf16)
                                kk = Kb
                                while kk > 1:
                                    half = kk // 2
                                    nc.vector.tensor_tensor(
                                        out=msgV[:, :, 0:half, :],
                                        in0=msgV[:, :, 0:half, :],
                                        in1=msgV[:, :, half:2 * half, :],
                                        op=OP.add)
                                    if kk % 2 == 1:
                                        nc.vector.tensor_tensor(
                                            out=msgV[:, :, 0:1, :],
                                            in0=msgV[:, :, 0:1, :],
                                            in1=msgV[:, :, kk - 1:kk, :],
                                            op=OP.add)
                                    kk = half
                                # pack partial rows ([width:] stays garbage —
                                # never read by the combine phase)
                                nc.vector.tensor_copy(
                                    pbV[:, 0:ng, 0:width],
                                    msgV[:, :, 0, :])
                                if KEDGE >= 3:
                                    nc.sync.dma_start(
                                        Ptab[:][int(PT_base[wi]):
                                                int(PT_base[wi]) + 128 * Gn, :]
                                        .rearrange("(p g) e -> p g e", p=128)
                                        [:, g0:g1, :],
                                        pbV[:, 0:ng, :])

                if STAGE >= 2:
                    edge_phase(Rw1, Ptab1, 1)

                # ------------- combine helpers -------------
                def combine_chunks(Ptab, body, cp, cxp, tagp, post=None,
                                   gcp=None):
                    ixcw = cxp.tile([128, NWIN * BLOCKS * 8], I16, tag="ixcw")
                    nc.sync.dma_start(ixcw[:], idx_c[:, :])
                    for b0 in range(0, BLOCKS, CHUNK):
                        b1 = min(b0 + CHUNK, BLOCKS)
                        nb = b1 - b0
                        CWs = []
                        for wi in range(NWIN):
                            nidx = nb * 128
                            off = (wi * BLOCKS + b0) * 128 // 16
                            ixc = ixcw[:, off: off + nidx // 16]
                            CW = (gcp or cp).tile(
                                [128, CHUNK * RW], BF,
                                tag=f"cw{tagp}{wi}")
                            _gi = nc.gpsimd.dma_gather(
                                out_ap=CW[:].rearrange(
                                    "p (b e) -> p b e", e=RW)[:, 0:nb, :],
                                in_ap=Ptab[:][int(PT_base[wi]):
                                              int(PT_base[wi]) + PT_rows[wi], :],
                                idxs_ap=ixc,
                                num_idxs=nidx, num_idxs_reg=nidx,
                                elem_size=RW, single_packet=KSP,
                                queue_num=qn[0] % 4)
                            if qn[1] is not None:
                                tile.add_dep_helper(_gi.ins, qn[1].ins, sync=False,
                                                    reason="swdge order")
                            qn[1] = _gi
                            qn[0] += 1
                            CWs.append(CW[:].rearrange("p (b e) -> p b e", e=RW))
                        body(b0, b1, CWs)
                        if post is not None:
                            post(b0 // CHUNK)

                def add4(cp, CWs, nb, lo, hi, ftag):
                    width = hi - lo
                    acc = cp.tile([128, CHUNK * width], F32, tag=f"acc{ftag}")
                    t0 = cp.tile([128, CHUNK * width], F32, tag=f"t0{ftag}")
                    accV = acc[:].rearrange("p (b f) -> p b f", f=width)
                    t0V = t0[:].rearrange("p (b f) -> p b f", f=width)
                    nc.vector.tensor_tensor(
                        out=accV[:, 0:nb], in0=CWs[0][:, 0:nb, lo:hi],
                        in1=CWs[1][:, 0:nb, lo:hi], op=OP.add)
                    if NWIN > 2:
                        nc.vector.tensor_tensor(
                            out=t0V[:, 0:nb], in0=CWs[2][:, 0:nb, lo:hi],
                            in1=CWs[3][:, 0:nb, lo:hi], op=OP.add)
                        nc.vector.tensor_tensor(
                            out=accV[:, 0:nb], in0=accV[:, 0:nb],
                            in1=t0V[:, 0:nb], op=OP.add)
                    return accV

                def add4_f32(cp, CWs, nb, lo, nf, ftag):
                    acc = cp.tile([128, CHUNK * nf], F32, tag=f"acs{ftag}")
                    t0 = cp.tile([128, CHUNK * nf], F32, tag=f"ts{ftag}")
                    accV = acc[:].rearrange("p (b f) -> p b f", f=nf)
                    t0V = t0[:].rearrange("p (b f) -> p b f", f=nf)
                    nc.vector.tensor_tensor(
                        out=accV[:, 0:nb],
                        in0=CWs[0][:, 0:nb, lo:lo + 2 * nf].bitcast(F32),
                        in1=CWs[1][:, 0:nb, lo:lo + 2 * nf].bitcast(F32),
                        op=OP.add)
                    if NWIN > 2:
                        nc.vector.tensor_tensor(
                            out=t0V[:, 0:nb],
                            in0=CWs[2][:, 0:nb, lo:lo + 2 * nf].bitcast(F32),
                            in1=CWs[3][:, 0:nb, lo:lo + 2 * nf].bitcast(F32),
                            op=OP.add)
                        nc.vector.tensor_tensor(
                            out=accV[:, 0:nb], in0=accV[:, 0:nb],
                            in1=t0V[:, 0:nb], op=OP.add)
                    return accV

                # ---------------- combine L1 + dense layer 2 ----------------
                if STAGE >= 3:
                  with (
                      tc.tile_pool(name="c1", bufs=2) as cp,
                      tc.tile_pool(name="c1g", bufs=2 + KCB) as gcp1,
                      tc.tile_pool(name="c1x", bufs=2) as cxp,
                      tc.tile_pool(name="c1p", bufs=2, space="PSUM") as cpp,
                  ):
                      def c1_body(b0, b1, CWs):
                          nb = b1 - b0
                          UcV = add4(cp, CWs, nb, 0, 72, "u1")
                          Ux = UcV.rearrange("p b (h x) -> p b h x", x=9)
                          rinv = cp.tile([128, CHUNK * H1], F32, tag="rinv")
                          nc.vector.tensor_scalar(
                              out=rinv[:][:, 0:nb * H1].rearrange(
                                  "p (b h) -> p b h", b=nb),
                              in0=Ux[:, :, :, 8], scalar1=EPS,
                              scalar2=None, op0=OP.add)
                          nc.vector.reciprocal(rinv[:, 0:nb * H1],
                                               rinv[:, 0:nb * H1])
                          o1c = cp.tile([128, CHUNK * HD1], F32, tag="o1c")
                          nc.vector.tensor_tensor(
                              out=o1c[:][:, 0:nb * HD1].rearrange(
                                  "p (b h d) -> p b h d", b=nb, h=H1),
                              in0=Ux[:, :, :, 0:D1],
                              in1=rinv[:][:, 0:nb * H1].rearrange(
                                  "p (b h) -> p b h", b=nb).unsqueeze(3)
                                  .to_broadcast([128, nb, H1, D1]),
                              op=OP.mult)
                          nc.vector.tensor_tensor(
                              out=o1c[:][:, 0:nb * HD1].rearrange(
                                  "p (b f) -> p b f", b=nb),
                              in0=o1c[:][:, 0:nb * HD1].rearrange(
                                  "p (b f) -> p b f", b=nb),
                              in1=reps["b1"][:].unsqueeze(1)
                                  .to_broadcast([128, nb, HD1]),
                              op=OP.add)
                          of = o1c[:, 0:nb * HD1]
                          mn = cp.tile([128, CHUNK * HD1], F32, tag="mn")
                          nc.vector.tensor_scalar(
                              out=mn[:, 0:nb * HD1], in0=of, scalar1=0.0,
                              scalar2=None, op0=OP.min)
                          ex = cp.tile([128, CHUNK * HD1], F32, tag="ex")
                          nc.scalar.activation(ex[:, 0:nb * HD1], mn[:, 0:nb * HD1],
                                               ACT.Exp)
                          nc.vector.tensor_scalar(
                              out=of, in0=of, scalar1=0.0, scalar2=None, op0=OP.max)
                          nc.vector.tensor_tensor(
                              out=of, in0=of, in1=ex[:, 0:nb * HD1], op=OP.add)
                          nc.vector.tensor_scalar(
                              out=of, in0=of, scalar1=-1.0, scalar2=None, op0=OP.add)
                          # h2 = elu @ W2 : transpose 2 blocks at a time
                          h2c = cp.tile([128, CHUNK * C], F32, tag="h2c")
                          h2cV = h2c[:][:, 0:nb * C].rearrange(
                              "p (b f) -> p b f", b=nb)
                          for bp in range(0, nb, 2):
                              npair = min(2, nb - bp)
                              tp = cpp.tile([128, 128], F32, tag="tp")
                              nc.tensor.transpose(
                                  out=tp[0:npair * HD1, :],
                                  in_=o1c[:, bp * HD1:(bp + npair) * HD1],
                                  identity=ident[:])
                              eT_ = cp.tile([128, 128], F32, tag="eT2")
                              nc.vector.tensor_copy(eT_[0:npair * HD1, :],
                                                    tp[0:npair * HD1, :])
                              for j in range(npair):
                                  h2p = cpp.tile([128, C], F32, tag="h2p")
                                  nc.tensor.matmul(
                                      out=h2p[:],
                                      lhsT=eT_[j * HD1:(j + 1) * HD1, :],
                                      rhs=w2_t[j * HD1:(j + 1) * HD1, :],
                                      start=True, stop=True)
                                  nc.vector.tensor_copy(
                                      h2c[:, (bp + j) * C:(bp + j + 1) * C],
                                      h2p[:])
                          # attention logits for layer 2 (chunk-wide)
                          tmp2 = cp.tile([128, CHUNK * C], F32, tag="tmp2")
                          a2s = cp.tile([128, CHUNK], F32, tag="a2s")
                          a2d = cp.tile([128, CHUNK], F32, tag="a2d")
                          for nm, dst in (("atts2", a2s), ("attd2", a2d)):
                              nc.vector.tensor_tensor(
                                  out=tmp2[:][:, 0:nb * C].rearrange(
                                      "p (b f) -> p b f", b=nb),
                                  in0=h2cV,
                                  in1=reps[nm][:].unsqueeze(1)
                                      .to_broadcast([128, nb, C]),
                                  op=OP.mult)
                              nc.vector.tensor_reduce(
                                  out=dst[:, 0:nb],
                                  in_=tmp2[:][:, 0:nb * C].rearrange(
                                      "p (b f) -> p b f", b=nb),
                                  axis=AX, op=OP.add)
                          row2c = cp.tile([128, CHUNK * RW], BF, tag="row2c")
                          r2V = row2c[:].rearrange("p (b e) -> p b e", e=RW)
                          nc.vector.tensor_copy(r2V[:, 0:nb, 0:C], h2cV)
                          nc.vector.memset(r2V[:, 0:nb, 40:41], 1.0)
                          nc.vector.tensor_copy(
                              r2V[:, 0:nb, 41:42],
                              a2s[:][:, 0:nb].unsqueeze(2))
                          nc.sync.dma_start(
                              Rshard2[b0 * 128:b1 * 128, :].rearrange(
                                  "(b p) e -> p b e", p=128),
                              r2V[:, 0:nb, :])
                          adr = cp.tile([128, CHUNK], BF, tag="adr")
                          nc.vector.tensor_copy(adr[:, 0:nb], a2d[:, 0:nb])
                          nc.sync.dma_start(
                              AdstT[b0 * 128:b1 * 128, 8:9].rearrange(
                                  "(b p) e -> p b e", p=128),
                              adr[:][:, 0:nb].unsqueeze(2))

                      gz = cp.tile([128, 42], BF, tag="gz")
                      nc.vector.memset(gz[:, 0:41], 0.0)
                      nc.vector.memset(gz[:, 41:42], -100.0)

                      # one combine chunk == one quarter: ghost-fill the
                      # quarter and launch its layer-2 AllGather immediately,
                      # overlapping later chunks and edge-2 windows
                      def c1_post(q):
                          if STAGE < 4:
                              return
                          g0r = q * QROWS + QREAL
                          nc.sync.dma_start(
                              Rshard2[g0r:(q + 1) * QROWS, 0:42],
                              gz[: QROWS - QREAL, :])
                          if KAG == 1:
                              nc.gpsimd.collective_compute(
                                  "AllGather", OP.bypass, replica_groups=rg,
                                  ins=[Rshard2[:][q * QROWS:
                                                  (q + 1) * QROWS, :].opt()],
                                  outs=[Rw2[q].opt()])
                          elif KAG == 2:
                              nc.sync.dma_start(
                                  Rw2[q][:][0:128, :],
                                  Rshard2[q * QROWS:q * QROWS + 128, :])
                          else:
                              for c in range(NC):
                                  nc.sync.dma_start(
                                      Rw2[q][:][c * QROWS:(c + 1) * QROWS, :],
                                      Rshard2[q * QROWS:(q + 1) * QROWS, :])

                      combine_chunks(Ptab1, c1_body, cp, cxp, "a", post=c1_post,
                                     gcp=gcp1)

                if STAGE >= 4:
                    edge_phase(Rw2, Ptab2, 2)

                # ---------------- combine L2 + log_softmax ----------------
                if STAGE >= 5:
                  with (
                      tc.tile_pool(name="c2", bufs=2) as cp2,
                      tc.tile_pool(name="c2g", bufs=2 + KCB) as gcp2,
                      tc.tile_pool(name="c2x", bufs=2) as cxp2,
                  ):
                      def c2_body(b0, b1, CWs):
                          nb = b1 - b0
                          UcV = add4(cp2, CWs, nb, 0, C + 1, "u2")
                          rinv = cp2.tile([128, CHUNK], F32, tag="rinv2")
                          nc.vector.tensor_scalar(
                              out=rinv[:, 0:nb],
                              in0=UcV[:, :, C],
                              scalar1=EPS, scalar2=None, op0=OP.add)
                          nc.vector.reciprocal(rinv[:, 0:nb], rinv[:, 0:nb])
                          o2c = cp2.tile([128, CHUNK * C], F32, tag="o2c")
                          o2V = o2c[:][:, 0:nb * C].rearrange(
                              "p (b f) -> p b f", b=nb)
                          nc.vector.tensor_tensor(
                              out=o2V, in0=UcV[:, :, 0:C],
                              in1=rinv[:][:, 0:nb].unsqueeze(2)
                                  .to_broadcast([128, nb, C]),
                              op=OP.mult)
                          nc.vector.tensor_tensor(
                              out=o2V, in0=o2V,
                              in1=reps["b2"][:].unsqueeze(1)
                                  .to_broadcast([128, nb, C]),
                              op=OP.add)
                          mx = cp2.tile([128, CHUNK], F32, tag="mx")
                          nc.vector.tensor_reduce(
                              out=mx[:, 0:nb], in_=o2V, axis=AX, op=OP.max)
                          nc.vector.tensor_tensor(
                              out=o2V, in0=o2V,
                              in1=mx[:][:, 0:nb].unsqueeze(2)
                                  .to_broadcast([128, nb, C]),
                              op=OP.subtract)
                          ex2 = cp2.tile([128, CHUNK * C], F32, tag="ex2")
                          nc.scalar.activation(ex2[:, 0:nb * C], o2c[:, 0:nb * C],
                                               ACT.Exp)
                          ss = cp2.tile([128, CHUNK], F32, tag="ss")
                          nc.vector.tensor_reduce(
                              out=ss[:, 0:nb],
                              in_=ex2[:][:, 0:nb * C].rearrange(
                                  "p (b f) -> p b f", b=nb),
                              axis=AX, op=OP.add)
                          nc.scalar.activation(ss[:, 0:nb], ss[:, 0:nb], ACT.Ln)
                          nc.vector.tensor_tensor(
                              out=o2V, in0=o2V,
                              in1=ss[:][:, 0:nb].unsqueeze(2)
                                  .to_broadcast([128, nb, C]),
                              op=OP.subtract)
                          nc.sync.dma_start(
                              out[b0 * 128:b1 * 128, :].rearrange(
                                  "(b p) c -> p b c", p=128),
                              o2V)

                      combine_chunks(Ptab2, c2_body, cp2, cxp2, "b", gcp=gcp2)

    nc.compile()
    return nc


_CACHE = {}


def _in_maps(inputs, idx_e, idx_a, idx_c):
    x = np.asarray(inputs["x"], np.float32)
    maps = []
    for c in range(NC):
        xs = np.zeros((F_IN, SHARD), np.float32)
        for q in range(QN):
            xs[:, q * QROWS:q * QROWS + QREAL] = \
                x[c * REAL + q * QREAL:c * REAL + (q + 1) * QREAL].T
        maps.append({
            "xt": xs,
            "w1": np.asarray(inputs["W1"], np.float32),
            "w2": np.asarray(inputs["W2"], np.float32),
            "atts1": np.asarray(inputs["att_src1"], np.float32).reshape(1, HD1),
            "attd1": np.asarray(inputs["att_dst1"], np.float32).reshape(1, HD1),
            "b1": np.asarray(inputs["b1"], np.float32).reshape(1, HD1),
            "atts2": np.asarray(inputs["att_src2"], np.float32).reshape(1, C),
            "attd2": np.asarray(inputs["att_dst2"], np.float32).reshape(1, C),
            "b2": np.asarray(inputs["b2"], np.float32).reshape(1, C),
            "idx_e": idx_e[c], "idx_a": idx_a[c], "idx_c": idx_c[c],
        })
    return maps


def kernel(**inputs):
    from concourse import bass_utils

    struct, idx_e, idx_a, idx_c = preprocess(inputs["edge_index"])
    key = (struct["idx_e_w"], struct["idx_a_w"], struct["idx_c_w"],
           tuple(struct["G_w"]))
    if key not in _CACHE:
        _CACHE[key] = build(struct)
    nc = _CACHE[key]

    maps = _in_maps(inputs, idx_e, idx_a, idx_c)
    res = bass_utils.run_bass_kernel_spmd(nc, maps, core_ids=list(range(NC)))
    out = np.concatenate(
        [res.results[c]["out"].reshape(QN, QROWS, C)[:, :QREAL].reshape(-1, C)
         for c in range(NC)], axis=0)
    return out.astype(np.float32)


if __name__ == "__main__":
    rng = np.random.default_rng(0)
    ei = np.stack([rng.integers(0, N, E), rng.integers(0, N, E)])
    struct, *_ = preprocess(ei.astype(np.int64))
    print("G_w:", struct["G_w"])
    print("K sums:", [int(k.sum()) for k in struct["K_w"]])
    print("widths:", struct["idx_e_w"], struct["idx_a_w"], struct["idx_c_w"])



# revision 19
# speedup vs baseline: 1.4489x; 1.4489x over previous
"""GAT (2-layer graph attention) on 8 Trainium2 NeuronCores.

Node tables (256B rows) are AllGather'd and per-edge rows fetched with
dma_gather. int16 gather indices cap tables at 32768 rows; the rank space
is QUARTER-MAJOR (local node r -> quarter r // QREAL), so each of the 4
windows equals one quarter of every core's shard and each layer's AllGather
splits into 4 quarter-AGs launched as their producer chunk (25 dense blocks
/ 1 combine chunk) finishes — collectives pipeline with dense/combine and
with the edge-phase windows, and window stats balance (G_w uniform, less
K-padding). Each (dst-node, window) pair is a degree-sorted "virtual row"
producing partial softmax sums, combined by a second gather round. Softmax
max-subtraction is dropped (shift-invariant; logits are O(1)).

Row layout v2: layer-1 head h occupies [9h:9h+9) = 8 features + a baked
1.0 column (layer 2: h2[0:40] | 1.0@40 | asrc@41), so the attention-softmax
denominator falls out of the same bf16 msg-multiply + k-tree-reduce as the
weighted message — no separate strided reduce, no f32 packing, no memsets.
"""

import os as _os

import numpy as np
import ml_dtypes

bf16 = ml_dtypes.bfloat16

# ---------------- problem constants -----------------------------------
N = 100000
E = 1600000
NC = 8
F_IN = 512
H1, D1 = 8, 8
HD1 = H1 * D1
C = 40
NEG_SLOPE = 0.2
EPS = 1e-16

REAL = N // NC
BLOCKS = 100
SHARD = BLOCKS * 128
RANKS = NC * SHARD
# Quarter-major rank space: local node r lives in quarter q = r // QREAL at
# padded row q*QROWS + r % QREAL; window w == quarter w across all cores, so
# each layer's AllGather splits into 4 independent quarter-AGs that pipeline
# with the dense/combine producers and the edge-phase consumers.
QN = 4
QROWS = SHARD // QN          # 3200 padded rows per (core, quarter)
QREAL = REAL // QN           # 3125 real rows per (core, quarter)
WIN = NC * QROWS             # 25600 <= 32767 (int16 gather indices)
NWIN = QN
RW = 128                 # bf16 elems per table row (256B)
BATCH_KMAX = int(_os.environ.get("KBM", "64"))  # max sum-of-K per gather call
CHUNK = 25               # combine blocks per chunk
ADST_GMAX = int(_os.environ.get("KADG", "64"))


def _wrap_idx(flat):
    n = flat.shape[0]
    assert n % 16 == 0
    w16 = flat.reshape(n // 16, 16).T
    return np.tile(w16, (8, 1)).astype(np.int16)


# Pad slots must NOT all hit one ghost row: 17% of descriptors hammering a
# single 256B HBM row serializes one bank and slows the whole gather stream
# ~2.8x (5.3 vs 1.9 ns/desc measured). Spread over all NC*(QROWS-QREAL)
# ghost rows.
GHOST_ROWS = np.concatenate(
    [c * QROWS + QREAL + np.arange(QROWS - QREAL) for c in range(NC)])


def preprocess(edge_index):
    src = np.asarray(edge_index[0], np.int64)
    dst = np.asarray(edge_index[1], np.int64)
    loops = np.arange(N, dtype=np.int64)
    src = np.concatenate([src, loops])
    dst = np.concatenate([dst, loops])

    core = dst // REAL
    rl = dst % REAL
    r_local = (rl // QREAL) * QROWS + rl % QREAL      # padded dst row
    src_c = src // REAL
    src_r = src % REAL
    w = src_r // QREAL                                 # window == quarter
    rel = src_c * QROWS + src_r % QREAL                # rank within window

    key = (core * NWIN + w) * SHARD + r_local
    deg = np.bincount(key, minlength=NC * NWIN * SHARD).reshape(
        NC, NWIN, SHARD)

    vlists = {}
    nnz = np.zeros((NC, NWIN), int)
    for c in range(NC):
        for wi in range(NWIN):
            d = deg[c, wi]
            rs = np.nonzero(d)[0]
            order = np.argsort(-d[rs], kind="stable")
            rs = rs[order]
            vlists[(c, wi)] = (rs, d[rs])
            nnz[c, wi] = len(rs)

    G_w = [max(1, int(np.ceil(nnz[:, wi].max() / 128))) for wi in range(NWIN)]
    K_w = []
    for wi in range(NWIN):
        ks = np.zeros(G_w[wi], int)
        for c in range(NC):
            degs = vlists[(c, wi)][1]
            for g in range(G_w[wi]):
                if g * 128 < len(degs):
                    ks[g] = max(ks[g], degs[g * 128])
        ks = np.maximum(ks, 1)
        K_w.append(ks)

    # K-uniform batches: every group in a batch is padded to the batch max K
    # so per-group DVE ops fuse into single per-batch ops.
    batches_w = []
    for wi in range(NWIN):
        batches = []
        g0 = 0
        while g0 < G_w[wi]:
            kb = int(K_w[wi][g0])          # groups sorted by K desc
            nb = max(1, min(BATCH_KMAX // kb, G_w[wi] - g0))
            g1 = g0 + nb
            K_w[wi][g0:g1] = kb            # pad K uniform within batch
            batches.append((g0, g1, int(kb * nb)))
            g0 = g1
        batches_w.append(batches)

    cumK_w = [np.concatenate([[0], np.cumsum(K_w[wi])]) for wi in range(NWIN)]

    sortpos = np.full((NC, NWIN, SHARD), -1, np.int64)
    for c in range(NC):
        for wi in range(NWIN):
            rs = vlists[(c, wi)][0]
            sortpos[c, wi, rs] = np.arange(len(rs))
    vpos = sortpos[core, w, r_local]
    order = np.argsort(key, kind="stable")
    k_in_row = np.empty(len(key), np.int64)
    sk_ = key[order]
    first = np.concatenate([[True], sk_[1:] != sk_[:-1]])
    starts = np.nonzero(first)[0]
    run_id = np.cumsum(first) - 1
    k_in_row[order] = np.arange(len(key)) - starts[run_id]

    gv = vpos // 128
    pv = vpos % 128

    idx_e_cores, idx_a_cores, idx_c_cores = [], [], []
    for c in range(NC):
        e_parts = []
        m_c = core == c
        for wi in range(NWIN):
            ncol = int(cumK_w[wi][-1])
            A = GHOST_ROWS[np.arange(128 * ncol) % len(GHOST_ROWS)].reshape(
                128, ncol)
            m = m_c & (w == wi)
            col = cumK_w[wi][gv[m]] + k_in_row[m]
            A[pv[m], col] = rel[m]
            for (g0, g1, _sk) in batches_w[wi]:
                c0, c1 = int(cumK_w[wi][g0]), int(cumK_w[wi][g1])
                e_parts.append(_wrap_idx(A[:, c0:c1].T.reshape(-1)))
        idx_e_cores.append(np.concatenate(e_parts, axis=1))

        a_parts = []
        for wi in range(NWIN):
            rs = vlists[(c, wi)][0]
            R_flat = np.zeros(G_w[wi] * 128, np.int64)
            R_flat[: len(rs)] = rs
            R = R_flat.reshape(G_w[wi], 128).T
            g0 = 0
            while g0 < G_w[wi]:
                g1 = min(g0 + ADST_GMAX, G_w[wi])
                a_parts.append(_wrap_idx(R[:, g0:g1].T.reshape(-1)))
                g0 = g1
        idx_a_cores.append(np.concatenate(a_parts, axis=1))

        c_parts = []
        for wi in range(NWIN):
            zr = 128 * G_w[wi]
            # spread no-virtual-row pads over 128 zero rows (same bank-
            # hammering fix as GHOST_ROWS)
            Cidx = zr + np.arange(SHARD, dtype=np.int64) % 128
            rs = vlists[(c, wi)][0]
            vp = np.arange(len(rs))
            Cidx[rs] = (vp % 128) * G_w[wi] + (vp // 128)
            Cm = Cidx.reshape(BLOCKS, 128).T
            for b0 in range(0, BLOCKS, CHUNK):
                b1 = min(b0 + CHUNK, BLOCKS)
                c_parts.append(_wrap_idx(Cm[:, b0:b1].T.reshape(-1)))
        idx_c_cores.append(np.concatenate(c_parts, axis=1))

    struct = dict(
        G_w=G_w, K_w=K_w, batches_w=batches_w, cumK_w=cumK_w,
        idx_e_w=idx_e_cores[0].shape[1], idx_a_w=idx_a_cores[0].shape[1],
        idx_c_w=idx_c_cores[0].shape[1],
    )
    return struct, idx_e_cores, idx_a_cores, idx_c_cores


# -----------------------------------------------------------------------
def build(struct):
    import os
    STAGE = int(os.environ.get("KSTAGE", "5"))
    KEDGE = int(os.environ.get("KEDGE", "3"))
    KREPS = int(os.environ.get("KREPS", "1"))
    KAG = int(os.environ.get("KAG", "1"))          # 0: fake AG with local copy
    # 2: skip AG entirely (timing-only; tables hold garbage)
    KADST = int(os.environ.get("KADST", "1"))      # 0: skip adst gathers
    # 1: AllGather only the used row columns (88 for L1, 42 for L2) via
    # strided APs — cuts AG wire bytes ~45%
    KAGW = int(os.environ.get("KAGW", "0"))
    # 0: Local AG outputs (fastest with pipelined quarter-AGs; >4 Shared
    # collectives also crash the runtime), 1: all Shared, 2: layer-1 Shared
    KSHARED = int(os.environ.get("KSHARED", "0"))
    KSP = bool(int(os.environ.get("KSP", "0")))    # single_packet for gathers
    # 1: leaky-relu on ACT engine (removes 2 DVE ops/batch, ~290 us; HW
    # Lrelu numerics are looser — rel err 1.4e-3 vs 4.8e-5, gate is 2e-2)
    KLR = int(os.environ.get("KLR", "1"))
    # 1: use Prelu instead of Lrelu — same math (x>0 ? x : alpha*x) but
    # Prelu shares the act-table set with Exp, killing ~126 table reloads
    KPRELU = int(os.environ.get("KPRELU", "1"))
    KEB = int(os.environ.get("KEB", "0"))          # 1: bf16 edge logits
    KCB = int(os.environ.get("KCB", "1"))          # deeper combine-gather
    # staging (cw tiles own pool): prefetch next chunk, -375 us
    KBUF = int(os.environ.get("KBUF", "0"))        # 1: trade a gather buf
    # for deeper compute pools (gp 3, ep/epb 3, epm 2); 0 wins now that
    # gathers are fast (ghost-spread fix): 3.60 vs 3.85 ms
    # gather-pool depth: deeper keeps more dma_gathers in flight (4->2.9ms,
    # 7->2.66ms); 7x2MB fits SBUF alongside the edge compute pools
    KGB = int(os.environ.get("KGB", "7"))
    import concourse.bacc as bacc
    import concourse.mybir as mybir
    import concourse.tile as tile
    from concourse.masks import make_identity

    F32 = mybir.dt.float32
    BF = mybir.dt.bfloat16
    I16 = mybir.dt.int16
    AX = mybir.AxisListType.X
    OP = mybir.AluOpType
    ACT = mybir.ActivationFunctionType

    G_w, K_w, batches_w = struct["G_w"], struct["K_w"], struct["batches_w"]
    cumK_w = struct["cumK_w"]
    KMAX = int(max(max(k) for k in K_w))
    FC = F_IN // 128

    nc = bacc.Bacc("TRN2", target_bir_lowering=False, debug=False,
                   num_devices=NC, num_swdge_queues=4)

    xt = nc.dram_tensor("xt", [F_IN, SHARD], F32, kind="ExternalInput").ap()
    w1 = nc.dram_tensor("w1", [F_IN, HD1], F32, kind="ExternalInput").ap()
    w2 = nc.dram_tensor("w2", [HD1, C], F32, kind="ExternalInput").ap()
    vec_in = {}
    for nm, width in [("atts1", HD1), ("attd1", HD1), ("b1", HD1),
                      ("atts2", C), ("attd2", C), ("b2", C)]:
        vec_in[nm] = nc.dram_tensor(nm, [1, width], F32,
                                    kind="ExternalInput").ap()
    idx_e = nc.dram_tensor("idx_e", [128, struct["idx_e_w"]], I16,
                           kind="ExternalInput").ap()
    idx_a = nc.dram_tensor("idx_a", [128, struct["idx_a_w"]], I16,
                           kind="ExternalInput").ap()
    idx_c = nc.dram_tensor("idx_c", [128, struct["idx_c_w"]], I16,
                           kind="ExternalInput").ap()
    out = nc.dram_tensor("out", [SHARD, C], F32, kind="ExternalOutput").ap()

    rg = [list(range(NC))]
    PT_rows = [128 * G_w[wi] + 128 for wi in range(NWIN)]
    PT_total = sum(PT_rows)
    PT_base = np.concatenate([[0], np.cumsum(PT_rows)]).astype(int)

    with tile.TileContext(nc) as tc:
        for _rep in range(KREPS):
            with (
                tc.tile_pool(name="dram", bufs=1, space="DRAM") as dpool,
                tc.tile_pool(name="setup", bufs=1) as sup,
                tc.tile_pool(name="psum0", bufs=2, space="PSUM") as psp,
            ):
                Rshard1 = dpool.tile([SHARD, RW], BF, tag="rs1")
                Rshard2 = dpool.tile([SHARD, RW], BF, tag="rs2")
                # KSHARED: 0 = all Local, 1 = all Shared, 2 = layer-1 only
                asp1 = "Shared" if KSHARED in (1, 2) else "Local"
                asp2 = "Shared" if KSHARED == 1 else "Local"
                Rw1 = [dpool.tile([WIN, RW], BF, tag=f"rf1{q}",
                                  addr_space=asp1, name=f"Rw1_{q}")
                       for q in range(QN)]
                Rw2 = [dpool.tile([WIN, RW], BF, tag=f"rf2{q}",
                                  addr_space=asp2, name=f"Rw2_{q}")
                       for q in range(QN)]
                AdstT = dpool.tile([SHARD, RW], BF, tag="adt")
                Ptab1 = dpool.tile([PT_total, RW], BF, tag="pt1")
                Ptab2 = dpool.tile([PT_total, RW], BF, tag="pt2")

                ident = sup.tile([128, 128], F32)
                make_identity(nc, ident[:])
                ones_row = sup.tile([1, 128], F32)
                nc.vector.memset(ones_row[:], 1.0)

                w1_t = sup.tile([128, FC * HD1], F32)
                nc.sync.dma_start(
                    w1_t[:].rearrange("p (c n) -> p c n", c=FC),
                    w1.rearrange("(c p) n -> p c n", p=128),
                )
                w2_t = sup.tile([128, C], F32)
                nc.sync.dma_start(w2_t[0:HD1, :], w2[:, :])
                nc.sync.dma_start(w2_t[HD1:2 * HD1, :], w2[:, :])

                reps = {}
                for nm in ["atts1", "attd1", "b1", "atts2", "attd2", "b2"]:
                    width = HD1 if nm in ("atts1", "attd1", "b1") else C
                    v = sup.tile([1, width], F32, tag=f"v_{nm}")
                    nc.sync.dma_start(v[:], vec_in[nm][:, :])
                    ps = psp.tile([128, width], F32, tag="rep_ps")
                    nc.tensor.matmul(out=ps[:], lhsT=ones_row[:], rhs=v[:],
                                     start=True, stop=True)
                    r_ = sup.tile([128, width], F32, tag=f"rep_{nm}")
                    nc.vector.tensor_copy(r_[:], ps[:])
                    reps[nm] = r_

                ghost1 = sup.tile([128, 88], BF)
                nc.vector.memset(ghost1[:], 0.0)
                nc.vector.memset(ghost1[:, 72:80], -100.0)
                zrow = sup.tile([128, RW], BF)
                nc.vector.memset(zrow[:], 0.0)
                for wi in range(NWIN):
                    zr = int(PT_base[wi]) + 128 * G_w[wi]
                    nc.sync.dma_start(Ptab1[:][zr:zr + 128, :], zrow[:])
                    nc.sync.dma_start(Ptab2[:][zr:zr + 128, :], zrow[:])

                # ---------------- dense layer 1 ----------------
                with (
                    tc.tile_pool(name="d1", bufs=3) as dp,
                    tc.tile_pool(name="d1p", bufs=2, space="PSUM") as dpp,
                ):
                    for t in range(BLOCKS):
                        xtile = dp.tile([128, FC * 128], F32, tag="x")
                        nc.sync.dma_start(
                            xtile[:].rearrange("p (c n) -> p c n", c=FC),
                            xt.rearrange("(c p) n -> p c n", p=128)[
                                :, :, t * 128:(t + 1) * 128],
                        )
                        hps = dpp.tile([128, HD1], F32, tag="h")
                        for cc in range(FC):
                            nc.tensor.matmul(
                                out=hps[:],
                                lhsT=xtile[:].rearrange(
                                    "p (c n) -> p c n", c=FC)[:, cc, :],
                                rhs=w1_t[:].rearrange(
                                    "p (c n) -> p c n", c=FC)[:, cc, :],
                                start=(cc == 0), stop=(cc == FC - 1),
                            )
                        # row layout v2: head h at [9h:9h+8], 1.0 at 9h+8
                        # (embedded softmax-denominator column), asrc@72,
                        # adst@80; [88:] never read.
                        row = dp.tile([128, RW], BF, tag="row")
                        rowh = row[:][:, 0:72].rearrange(
                            "p (h x) -> p h x", x=9)
                        nc.vector.tensor_copy(
                            rowh[:, :, 0:8],
                            hps[:].rearrange("p (h d) -> p h d", h=H1))
                        nc.vector.memset(rowh[:, :, 8:9], 1.0)
                        asrc_f = dp.tile([128, H1], F32, tag="asrcf")
                        adst_f = dp.tile([128, H1], F32, tag="adstf")
                        tmp = dp.tile([128, HD1], F32, tag="tmp")
                        for nm, dst_ap in (("atts1", asrc_f), ("attd1", adst_f)):
                            nc.vector.tensor_tensor(
                                out=tmp[:], in0=hps[:], in1=reps[nm][:],
                                op=OP.mult)
                            nc.vector.tensor_reduce(
                                out=dst_ap[:],
                                in_=tmp[:].rearrange("p (h d) -> p h d", h=H1),
                                axis=AX, op=OP.add)
                        nc.vector.tensor_copy(row[:, 72:80], asrc_f[:])
                        nc.vector.tensor_copy(row[:, 80:88], adst_f[:])
                        nc.sync.dma_start(Rshard1[t * 128:(t + 1) * 128, :],
                                          row[:])
                        arow = dp.tile([128, 9], BF, tag="arow")
                        nc.vector.tensor_copy(arow[:, 0:8], adst_f[:])
                        nc.vector.memset(arow[:, 8:9], 0.0)
                        nc.sync.dma_start(AdstT[t * 128:(t + 1) * 128, 0:9],
                                          arow[:])

                        # quarter q done after its 25 blocks: write its ghost
                        # rows and launch its AllGather so transfer overlaps
                        # the remaining dense blocks and later edge windows
                        if (t + 1) % (QROWS // 128) == 0:
                            q = t // (QROWS // 128)
                            g0r = q * QROWS + QREAL
                            nc.sync.dma_start(
                                Rshard1[g0r:(q + 1) * QROWS, 0:88],
                                ghost1[: QROWS - QREAL, :])
                            if KAG == 1:
                                w1hi = 88 if KAGW else RW
                                nc.gpsimd.collective_compute(
                                    "AllGather", OP.bypass, replica_groups=rg,
                                    ins=[Rshard1[:][q * QROWS:(q + 1) * QROWS,
                                                    0:w1hi].opt()],
                                    outs=[Rw1[q][:][:, 0:w1hi].opt()])
                            elif KAG == 2:
                                nc.sync.dma_start(
                                    Rw1[q][:][0:128, :],
                                    Rshard1[q * QROWS:q * QROWS + 128, :])
                            else:
                                for c in range(NC):
                                    nc.sync.dma_start(
                                        Rw1[q][:][c * QROWS:
                                                  (c + 1) * QROWS, :],
                                        Rshard1[q * QROWS:(q + 1) * QROWS, :])

                # ---------------- edge phase ----------------
                qn = [0, None]

                def edge_phase(Rws, Ptab, layer):
                    e_col = 0
                    a_col = 0
                    # v2 rows: layer 1 packs head h at [9h:9h+9] (8 feats +
                    # const 1.0 denominator column), asrc@72, adst@0 in AdstT;
                    # layer 2: h2[0:40], 1.0@40, asrc@41, adst@8 in AdstT.
                    if layer == 1:
                        Hh, Xw, alo, dlo = H1, D1 + 1, 72, 0
                    else:
                        Hh, Xw, alo, dlo = 1, C + 1, 41, 8
                    width = Hh * Xw
                    for wi in range(NWIN):
                        Rwin = Rws[wi]
                        Gn = G_w[wi]
                        ecols_w = 8 * sum(sk for (_g0, _g1, sk) in batches_w[wi])
                        acols_w = Gn * 8
                        with (
                            tc.tile_pool(name=f"ad{layer}{wi}", bufs=1) as apool,
                            tc.tile_pool(name=f"eg{layer}{wi}",
                                         bufs=KGB or (4 - KBUF)) as gp,
                            tc.tile_pool(name=f"ep{layer}{wi}",
                                         bufs=2 + KBUF) as ep,
                            tc.tile_pool(name=f"em{layer}{wi}",
                                         bufs=1 + KBUF) as epm,
                            tc.tile_pool(name=f"eb{layer}{wi}",
                                         bufs=2 + KBUF) as epb,
                            tc.tile_pool(name=f"ix{layer}{wi}", bufs=1) as ixp,
                        ):
                            ixw = ixp.tile([128, ecols_w], I16, tag="ixw")
                            nc.sync.dma_start(ixw[:], idx_e[:, e_col: e_col + ecols_w])
                            ixaw = ixp.tile([128, acols_w], I16, tag="ixaw")
                            nc.sync.dma_start(ixaw[:], idx_a[:, a_col: a_col + acols_w])
                            ecol_loc = 0
                            acol_loc = 0
                            adstG = apool.tile([128, Gn * RW], BF, tag="adstG")
                            adstG_v = adstG[:].rearrange("p (g e) -> p g e", e=RW)
                            if not KADST:
                                nc.vector.memset(adstG[:], 0.0)
                            g0 = 0
                            while g0 < Gn:
                                g1 = min(g0 + ADST_GMAX, Gn)
                                nidx = (g1 - g0) * 128
                                if KADST:
                                    _gi = nc.gpsimd.dma_gather(
                                        out_ap=adstG_v[:, g0:g1, :],
                                        in_ap=AdstT[:, :],
                                        idxs_ap=ixaw[:, acol_loc: acol_loc + nidx // 16],
                                        num_idxs=nidx, num_idxs_reg=nidx,
                                        elem_size=RW, single_packet=KSP,
                                        queue_num=qn[0] % 4)
                                    if qn[1] is not None:
                                        tile.add_dep_helper(_gi.ins, qn[1].ins, sync=False,
                                                            reason="swdge order")
                                    qn[1] = _gi
                                    qn[0] += 1
                                a_col += nidx // 16
                                acol_loc += nidx // 16
                                g0 = g1

                            for bidx, (g0, g1, sk) in enumerate(batches_w[wi]):
                                Kb = int(K_w[wi][g0])
                                ng = g1 - g0
                                nidx = 128 * sk
                                ixe = ixw[:, ecol_loc: ecol_loc + nidx // 16]
                                e_col += nidx // 16
                                ecol_loc += nidx // 16
                                G = gp.tile([128, BATCH_KMAX * RW], BF, tag="G")
                                Gv = G[:].rearrange("p (k e) -> p k e", e=RW)
                                Gg = G[:][:, 0:sk * RW].rearrange(
                                    "p (g k e) -> p g k e", g=ng, k=Kb)
                                _gi = nc.gpsimd.dma_gather(
                                    out_ap=Gv[:, 0:sk, :],
                                    in_ap=Rwin[:, :],
                                    idxs_ap=ixe,
                                    num_idxs=nidx, num_idxs_reg=nidx,
                                    elem_size=RW, single_packet=KSP,
                                    queue_num=qn[0] % 4)
                                if qn[1] is not None:
                                    tile.add_dep_helper(_gi.ins, qn[1].ins, sync=False,
                                                        reason="swdge order")
                                qn[1] = _gi
                                qn[0] += 1

                                if KEDGE < 2:
                                    continue
                                EDT = BF if KEB else F32
                                eT = ep.tile([128, BATCH_KMAX * Hh], EDT,
                                             tag="eT")
                                eV = eT[:][:, 0:sk * Hh].rearrange(
                                    "p (g k h) -> p g k h", g=ng, k=Kb)
                                pT = ep.tile([128, BATCH_KMAX * Hh], BF, tag="pT")
                                pb = epb.tile([128, BATCH_KMAX * RW], BF, tag="pb")
                                pbV = pb[:].rearrange("p (g e) -> p g e", e=RW)

                                # e = a_src[src] + a_dst (one op per batch)
                                nc.vector.tensor_tensor(
                                    out=eV[:, :, :, :],
                                    in0=Gg[:, :, :, alo:alo + Hh],
                                    in1=adstG_v[:, g0:g1, dlo:dlo + Hh]
                                        .unsqueeze(2)
                                        .to_broadcast([128, ng, Kb, Hh]),
                                    op=OP.add)
                                # leaky relu + exp
                                if KLR:
                                    eL = ep.tile([128, BATCH_KMAX * Hh], EDT,
                                                 tag="eL")
                                    nc.scalar.activation(
                                        eL[:, : sk * Hh], eT[:, : sk * Hh],
                                        ACT.Prelu if KPRELU else ACT.Lrelu,
                                        alpha=NEG_SLOPE)
                                    nc.scalar.activation(
                                        pT[:, : sk * Hh], eL[:, : sk * Hh],
                                        ACT.Exp)
                                else:
                                    ee = ep.tile([128, BATCH_KMAX * Hh], EDT,
                                                 tag="ee")
                                    nc.vector.tensor_scalar_mul(
                                        ee[:, : sk * Hh], eT[:, : sk * Hh],
                                        NEG_SLOPE)
                                    nc.vector.tensor_tensor(
                                        out=eT[:, : sk * Hh],
                                        in0=eT[:, : sk * Hh],
                                        in1=ee[:, : sk * Hh], op=OP.max)
                                    nc.scalar.activation(
                                        pT[:, : sk * Hh], eT[:, : sk * Hh],
                                        ACT.Exp)

                                # msg = p * [h | 1] (bf16; denominator rides in
                                # the const-1 column, so no separate reduce)
                                msg = epm.tile([128, BATCH_KMAX * width], BF,
                                               tag="msg")
                                msgV = msg[:][:, 0:sk * width].rearrange(
                                    "p (g k f) -> p g k f", g=ng, k=Kb)
                                nc.vector.tensor_tensor(
                                    out=msg[:][:, 0:sk * width].rearrange(
                                        "p (k h x) -> p k h x", k=sk, h=Hh),
                                    in0=Gv[:, 0:sk, 0:width].rearrange(
                                        "p k (h x) -> p k h x", h=Hh),
                                    in1=pT[:][:, 0:sk * Hh].rearrange(
                                        "p (k h) -> p k h", h=Hh)
                                        .unsqueeze(3)
                                        .to_broadcast([128, sk, Hh, Xw]),
                                    op=OP.mult)
                                # tree-reduce over k (uniform Kb, bf16)
                                kk = Kb
                                while kk > 1:
                                    half = kk // 2
                                    nc.vector.tensor_tensor(
                                        out=msgV[:, :, 0:half, :],
                                        in0=msgV[:, :, 0:half, :],
                                        in1=msgV[:, :, half:2 * half, :],
                                        op=OP.add)
                                    if kk % 2 == 1:
                                        nc.vector.tensor_tensor(
                                            out=msgV[:, :, 0:1, :],
                                            in0=msgV[:, :, 0:1, :],
                                            in1=msgV[:, :, kk - 1:kk, :],
                                            op=OP.add)
                                    kk = half
                                # pack partial rows ([width:] stays garbage —
                                # never read by the combine phase)
                                nc.vector.tensor_copy(
                                    pbV[:, 0:ng, 0:width],
                                    msgV[:, :, 0, :])
                                if KEDGE >= 3:
                                    nc.sync.dma_start(
                                        Ptab[:][int(PT_base[wi]):
                                                int(PT_base[wi]) + 128 * Gn, :]
                                        .rearrange("(p g) e -> p g e", p=128)
                                        [:, g0:g1, :],
                                        pbV[:, 0:ng, :])

                if STAGE >= 2:
                    edge_phase(Rw1, Ptab1, 1)

                # ------------- combine helpers -------------
                def combine_chunks(Ptab, body, cp, cxp, tagp, post=None,
                                   gcp=None):
                    ixcw = cxp.tile([128, NWIN * BLOCKS * 8], I16, tag="ixcw")
                    nc.sync.dma_start(ixcw[:], idx_c[:, :])
                    for b0 in range(0, BLOCKS, CHUNK):
                        b1 = min(b0 + CHUNK, BLOCKS)
                        nb = b1 - b0
                        CWs = []
                        for wi in range(NWIN):
                            nidx = nb * 128
                            off = (wi * BLOCKS + b0) * 128 // 16
                            ixc = ixcw[:, off: off + nidx // 16]
                            CW = (gcp or cp).tile(
                                [128, CHUNK * RW], BF,
                                tag=f"cw{tagp}{wi}")
                            _gi = nc.gpsimd.dma_gather(
                                out_ap=CW[:].rearrange(
                                    "p (b e) -> p b e", e=RW)[:, 0:nb, :],
                                in_ap=Ptab[:][int(PT_base[wi]):
                                              int(PT_base[wi]) + PT_rows[wi], :],
                                idxs_ap=ixc,
                                num_idxs=nidx, num_idxs_reg=nidx,
                                elem_size=RW, single_packet=KSP,
                                queue_num=qn[0] % 4)
                            if qn[1] is not None:
                                tile.add_dep_helper(_gi.ins, qn[1].ins, sync=False,
                                                    reason="swdge order")
                            qn[1] = _gi
                            qn[0] += 1
                            CWs.append(CW[:].rearrange("p (b e) -> p b e", e=RW))
                        body(b0, b1, CWs)
                        if post is not None:
                            post(b0 // CHUNK)

                def add4(cp, CWs, nb, lo, hi, ftag):
                    width = hi - lo
                    acc = cp.tile([128, CHUNK * width], F32, tag=f"acc{ftag}")
                    t0 = cp.tile([128, CHUNK * width], F32, tag=f"t0{ftag}")
                    accV = acc[:].rearrange("p (b f) -> p b f", f=width)
                    t0V = t0[:].rearrange("p (b f) -> p b f", f=width)
                    nc.vector.tensor_tensor(
                        out=accV[:, 0:nb], in0=CWs[0][:, 0:nb, lo:hi],
                        in1=CWs[1][:, 0:nb, lo:hi], op=OP.add)
                    if NWIN > 2:
                        nc.vector.tensor_tensor(
                            out=t0V[:, 0:nb], in0=CWs[2][:, 0:nb, lo:hi],
                            in1=CWs[3][:, 0:nb, lo:hi], op=OP.add)
                        nc.vector.tensor_tensor(
                            out=accV[:, 0:nb], in0=accV[:, 0:nb],
                            in1=t0V[:, 0:nb], op=OP.add)
                    return accV

                def add4_f32(cp, CWs, nb, lo, nf, ftag):
                    acc = cp.tile([128, CHUNK * nf], F32, tag=f"acs{ftag}")
                    t0 = cp.tile([128, CHUNK * nf], F32, tag=f"ts{ftag}")
                    accV = acc[:].rearrange("p (b f) -> p b f", f=nf)
                    t0V = t0[:].rearrange("p (b f) -> p b f", f=nf)
                    nc.vector.tensor_tensor(
                        out=accV[:, 0:nb],
                        in0=CWs[0][:, 0:nb, lo:lo + 2 * nf].bitcast(F32),
                        in1=CWs[1][:, 0:nb, lo:lo + 2 * nf].bitcast(F32),
                        op=OP.add)
                    if NWIN > 2:
                        nc.vector.tensor_tensor(
                            out=t0V[:, 0:nb],
                            in0=CWs[2][:, 0:nb, lo:lo + 2 * nf].bitcast(F32),
                            in1=CWs[3][:, 0:nb, lo:lo + 2 * nf].bitcast(F32),
                            op=OP.add)
                        nc.vector.tensor_tensor(
                            out=accV[:, 0:nb], in0=accV[:, 0:nb],
                            in1=t0V[:, 0:nb], op=OP.add)
                    return accV

                # ---------------- combine L1 + dense layer 2 ----------------
                if STAGE >= 3:
                  with (
                      tc.tile_pool(name="c1", bufs=2) as cp,
                      tc.tile_pool(name="c1g", bufs=2 + KCB) as gcp1,
                      tc.tile_pool(name="c1x", bufs=2) as cxp,
                      tc.tile_pool(name="c1p", bufs=2, space="PSUM") as cpp,
                  ):
                      def c1_body(b0, b1, CWs):
                          nb = b1 - b0
                          UcV = add4(cp, CWs, nb, 0, 72, "u1")
                          Ux = UcV.rearrange("p b (h x) -> p b h x", x=9)
                          rinv = cp.tile([128, CHUNK * H1], F32, tag="rinv")
                          nc.vector.tensor_scalar(
                              out=rinv[:][:, 0:nb * H1].rearrange(
                                  "p (b h) -> p b h", b=nb),
                              in0=Ux[:, :, :, 8], scalar1=EPS,
                              scalar2=None, op0=OP.add)
                          nc.vector.reciprocal(rinv[:, 0:nb * H1],
                                               rinv[:, 0:nb * H1])
                          o1c = cp.tile([128, CHUNK * HD1], F32, tag="o1c")
                          nc.vector.tensor_tensor(
                              out=o1c[:][:, 0:nb * HD1].rearrange(
                                  "p (b h d) -> p b h d", b=nb, h=H1),
                              in0=Ux[:, :, :, 0:D1],
                              in1=rinv[:][:, 0:nb * H1].rearrange(
                                  "p (b h) -> p b h", b=nb).unsqueeze(3)
                                  .to_broadcast([128, nb, H1, D1]),
                              op=OP.mult)
                          nc.vector.tensor_tensor(
                              out=o1c[:][:, 0:nb * HD1].rearrange(
                                  "p (b f) -> p b f", b=nb),
                              in0=o1c[:][:, 0:nb * HD1].rearrange(
                                  "p (b f) -> p b f", b=nb),
                              in1=reps["b1"][:].unsqueeze(1)
                                  .to_broadcast([128, nb, HD1]),
                              op=OP.add)
                          of = o1c[:, 0:nb * HD1]
                          mn = cp.tile([128, CHUNK * HD1], F32, tag="mn")
                          nc.vector.tensor_scalar(
                              out=mn[:, 0:nb * HD1], in0=of, scalar1=0.0,
                              scalar2=None, op0=OP.min)
                          ex = cp.tile([128, CHUNK * HD1], F32, tag="ex")
                          nc.scalar.activation(ex[:, 0:nb * HD1], mn[:, 0:nb * HD1],
                                               ACT.Exp)
                          nc.vector.tensor_scalar(
                              out=of, in0=of, scalar1=0.0, scalar2=None, op0=OP.max)
                          nc.vector.tensor_tensor(
                              out=of, in0=of, in1=ex[:, 0:nb * HD1], op=OP.add)
                          nc.vector.tensor_scalar(
                              out=of, in0=of, scalar1=-1.0, scalar2=None, op0=OP.add)
                          # h2 = elu @ W2 : transpose 2 blocks at a time
                          h2c = cp.tile([128, CHUNK * C], F32, tag="h2c")
                          h2cV = h2c[:][:, 0:nb * C].rearrange(
                              "p (b f) -> p b f", b=nb)
                          for bp in range(0, nb, 2):
                              npair = min(2, nb - bp)
                              tp = cpp.tile([128, 128], F32, tag="tp")
                              nc.tensor.transpose(
                                  out=tp[0:npair * HD1, :],
                                  in_=o1c[:, bp * HD1:(bp + npair) * HD1],
                                  identity=ident[:])
                              eT_ = cp.tile([128, 128], F32, tag="eT2")
                              nc.vector.tensor_copy(eT_[0:npair * HD1, :],
                                                    tp[0:npair * HD1, :])
                              for j in range(npair):
                                  h2p = cpp.tile([128, C], F32, tag="h2p")
                                  nc.tensor.matmul(
                                      out=h2p[:],
                                      lhsT=eT_[j * HD1:(j + 1) * HD1, :],
                                      rhs=w2_t[j * HD1:(j + 1) * HD1, :],
                                      start=True, stop=True)
                                  nc.vector.tensor_copy(
                                      h2c[:, (bp + j) * C:(bp + j + 1) * C],
                                      h2p[:])
                          # attention logits for layer 2 (chunk-wide)
                          tmp2 = cp.tile([128, CHUNK * C], F32, tag="tmp2")
                          a2s = cp.tile([128, CHUNK], F32, tag="a2s")
                          a2d = cp.tile([128, CHUNK], F32, tag="a2d")
                          for nm, dst in (("atts2", a2s), ("attd2", a2d)):
                              nc.vector.tensor_tensor(
                                  out=tmp2[:][:, 0:nb * C].rearrange(
                                      "p (b f) -> p b f", b=nb),
                                  in0=h2cV,
                                  in1=reps[nm][:].unsqueeze(1)
                                      .to_broadcast([128, nb, C]),
                                  op=OP.mult)
                              nc.vector.tensor_reduce(
                                  out=dst[:, 0:nb],
                                  in_=tmp2[:][:, 0:nb * C].rearrange(
                                      "p (b f) -> p b f", b=nb),
                                  axis=AX, op=OP.add)
                          row2c = cp.tile([128, CHUNK * RW], BF, tag="row2c")
                          r2V = row2c[:].rearrange("p (b e) -> p b e", e=RW)
                          nc.vector.tensor_copy(r2V[:, 0:nb, 0:C], h2cV)
                          nc.vector.memset(r2V[:, 0:nb, 40:41], 1.0)
                          nc.vector.tensor_copy(
                              r2V[:, 0:nb, 41:42],
                              a2s[:][:, 0:nb].unsqueeze(2))
                          nc.sync.dma_start(
                              Rshard2[b0 * 128:b1 * 128, :].rearrange(
                                  "(b p) e -> p b e", p=128),
                              r2V[:, 0:nb, :])
                          adr = cp.tile([128, CHUNK], BF, tag="adr")
                          nc.vector.tensor_copy(adr[:, 0:nb], a2d[:, 0:nb])
                          nc.sync.dma_start(
                              AdstT[b0 * 128:b1 * 128, 8:9].rearrange(
                                  "(b p) e -> p b e", p=128),
                              adr[:][:, 0:nb].unsqueeze(2))

                      gz = cp.tile([128, 42], BF, tag="gz")
                      nc.vector.memset(gz[:, 0:41], 0.0)
                      nc.vector.memset(gz[:, 41:42], -100.0)

                      # one combine chunk == one quarter: ghost-fill the
                      # quarter and launch its layer-2 AllGather immediately,
                      # overlapping later chunks and edge-2 windows
                      def c1_post(q):
                          if STAGE < 4:
                              return
                          g0r = q * QROWS + QREAL
                          nc.sync.dma_start(
                              Rshard2[g0r:(q + 1) * QROWS, 0:42],
                              gz[: QROWS - QREAL, :])
                          if KAG == 1:
                              w2hi = 42 if KAGW else RW
                              nc.gpsimd.collective_compute(
                                  "AllGather", OP.bypass, replica_groups=rg,
                                  ins=[Rshard2[:][q * QROWS:(q + 1) * QROWS,
                                                  0:w2hi].opt()],
                                  outs=[Rw2[q][:][:, 0:w2hi].opt()])
                          elif KAG == 2:
                              nc.sync.dma_start(
                                  Rw2[q][:][0:128, :],
                                  Rshard2[q * QROWS:q * QROWS + 128, :])
                          else:
                              for c in range(NC):
                                  nc.sync.dma_start(
                                      Rw2[q][:][c * QROWS:(c + 1) * QROWS, :],
                                      Rshard2[q * QROWS:(q + 1) * QROWS, :])

                      combine_chunks(Ptab1, c1_body, cp, cxp, "a", post=c1_post,
                                     gcp=gcp1)

                if STAGE >= 4:
                    edge_phase(Rw2, Ptab2, 2)

                # ---------------- combine L2 + log_softmax ----------------
                if STAGE >= 5:
                  with (
                      tc.tile_pool(name="c2", bufs=2) as cp2,
                      tc.tile_pool(name="c2g", bufs=2 + KCB) as gcp2,
                      tc.tile_pool(name="c2x", bufs=2) as cxp2,
                  ):
                      def c2_body(b0, b1, CWs):
                          nb = b1 - b0
                          UcV = add4(cp2, CWs, nb, 0, C + 1, "u2")
                          rinv = cp2.tile([128, CHUNK], F32, tag="rinv2")
                          nc.vector.tensor_scalar(
                              out=rinv[:, 0:nb],
                              in0=UcV[:, :, C],
                              scalar1=EPS, scalar2=None, op0=OP.add)
                          nc.vector.reciprocal(rinv[:, 0:nb], rinv[:, 0:nb])
                          o2c = cp2.tile([128, CHUNK * C], F32, tag="o2c")
                          o2V = o2c[:][:, 0:nb * C].rearrange(
                              "p (b f) -> p b f", b=nb)
                          nc.vector.tensor_tensor(
                              out=o2V, in0=UcV[:, :, 0:C],
                              in1=rinv[:][:, 0:nb].unsqueeze(2)
                                  .to_broadcast([128, nb, C]),
                              op=OP.mult)
                          nc.vector.tensor_tensor(
                              out=o2V, in0=o2V,
                              in1=reps["b2"][:].unsqueeze(1)
                                  .to_broadcast([128, nb, C]),
                              op=OP.add)
                          mx = cp2.tile([128, CHUNK], F32, tag="mx")
                          nc.vector.tensor_reduce(
                              out=mx[:, 0:nb], in_=o2V, axis=AX, op=OP.max)
                          nc.vector.tensor_tensor(
                              out=o2V, in0=o2V,
                              in1=mx[:][:, 0:nb].unsqueeze(2)
                                  .to_broadcast([128, nb, C]),
                              op=OP.subtract)
                          ex2 = cp2.tile([128, CHUNK * C], F32, tag="ex2")
                          nc.scalar.activation(ex2[:, 0:nb * C], o2c[:, 0:nb * C],
                                               ACT.Exp)
                          ss = cp2.tile([128, CHUNK], F32, tag="ss")
                          nc.vector.tensor_reduce(
                              out=ss[:, 0:nb],
                              in_=ex2[:][:, 0:nb * C].rearrange(
                                  "p (b f) -> p b f", b=nb),
                              axis=AX, op=OP.add)
                          nc.scalar.activation(ss[:, 0:nb], ss[:, 0:nb], ACT.Ln)
                          nc.vector.tensor_tensor(
                              out=o2V, in0=o2V,
                              in1=ss[:][:, 0:nb].unsqueeze(2)
                                  .to_broadcast([128, nb, C]),
                              op=OP.subtract)
                          nc.sync.dma_start(
                              out[b0 * 128:b1 * 128, :].rearrange(
                                  "(b p) c -> p b c", p=128),
                              o2V)

                      combine_chunks(Ptab2, c2_body, cp2, cxp2, "b", gcp=gcp2)

    nc.compile()
    return nc


_CACHE = {}


def _in_maps(inputs, idx_e, idx_a, idx_c):
    x = np.asarray(inputs["x"], np.float32)
    maps = []
    for c in range(NC):
        xs = np.zeros((F_IN, SHARD), np.float32)
        for q in range(QN):
            xs[:, q * QROWS:q * QROWS + QREAL] = \
                x[c * REAL + q * QREAL:c * REAL + (q + 1) * QREAL].T
        maps.append({
            "xt": xs,
            "w1": np.asarray(inputs["W1"], np.float32),
            "w2": np.asarray(inputs["W2"], np.float32),
            "atts1": np.asarray(inputs["att_src1"], np.float32).reshape(1, HD1),
            "attd1": np.asarray(inputs["att_dst1"], np.float32).reshape(1, HD1),
            "b1": np.asarray(inputs["b1"], np.float32).reshape(1, HD1),
            "atts2": np.asarray(inputs["att_src2"], np.float32).reshape(1, C),
            "attd2": np.asarray(inputs["att_dst2"], np.float32).reshape(1, C),
            "b2": np.asarray(inputs["b2"], np.float32).reshape(1, C),
            "idx_e": idx_e[c], "idx_a": idx_a[c], "idx_c": idx_c[c],
        })
    return maps


def kernel(**inputs):
    from concourse import bass_utils

    struct, idx_e, idx_a, idx_c = preprocess(inputs["edge_index"])
    key = (struct["idx_e_w"], struct["idx_a_w"], struct["idx_c_w"],
           tuple(struct["G_w"]))
    if key not in _CACHE:
        _CACHE[key] = build(struct)
    nc = _CACHE[key]

    maps = _in_maps(inputs, idx_e, idx_a, idx_c)
    res = bass_utils.run_bass_kernel_spmd(nc, maps, core_ids=list(range(NC)))
    out = np.concatenate(
        [res.results[c]["out"].reshape(QN, QROWS, C)[:, :QREAL].reshape(-1, C)
         for c in range(NC)], axis=0)
    return out.astype(np.float32)


if __name__ == "__main__":
    rng = np.random.default_rng(0)
    ei = np.stack([rng.integers(0, N, E), rng.integers(0, N, E)])
    struct, *_ = preprocess(ei.astype(np.int64))
    print("G_w:", struct["G_w"])
    print("K sums:", [int(k.sum()) for k in struct["K_w"]])
    print("widths:", struct["idx_e_w"], struct["idx_a_w"], struct["idx_c_w"])



# revision 31
# speedup vs baseline: 1.8363x; 1.2674x over previous
"""GAT (2-layer graph attention) on 8 Trainium2 NeuronCores.

Node tables (256B rows) are AllGather'd and per-edge rows fetched with
dma_gather. int16 gather indices cap tables at 32768 rows; the rank space
is QUARTER-MAJOR (local node r -> quarter r // QREAL), so each of the 4
windows equals one quarter of every core's shard and each layer's AllGather
splits into 4 quarter-AGs launched as their producer chunk (25 dense blocks
/ 1 combine chunk) finishes — collectives pipeline with dense/combine and
with the edge-phase windows, and window stats balance (G_w uniform, less
K-padding). Each (dst-node, window) pair is a degree-sorted "virtual row"
producing partial softmax sums, combined by a second gather round. Softmax
max-subtraction is dropped (shift-invariant; logits are O(1)).

Row layout v2: layer-1 head h occupies [9h:9h+9) = 8 features + a baked
1.0 column (layer 2: h2[0:40] | 1.0@40 | asrc@41), so the attention-softmax
denominator falls out of the same bf16 msg-multiply + k-tree-reduce as the
weighted message — no separate strided reduce, no f32 packing, no memsets.
"""

import os as _os

import numpy as np
import ml_dtypes

bf16 = ml_dtypes.bfloat16

# ---------------- problem constants -----------------------------------
N = 100000
E = 1600000
NC = 8
F_IN = 512
H1, D1 = 8, 8
HD1 = H1 * D1
C = 40
NEG_SLOPE = 0.2
EPS = 1e-16

REAL = N // NC
BLOCKS = 100
SHARD = BLOCKS * 128
RANKS = NC * SHARD
# Quarter-major rank space: local node r lives in quarter q = r // QREAL at
# padded row q*QROWS + r % QREAL; window w == quarter w across all cores, so
# each layer's AllGather splits into 4 independent quarter-AGs that pipeline
# with the dense/combine producers and the edge-phase consumers.
QN = 4
QROWS = SHARD // QN          # 3200 padded rows per (core, quarter)
QREAL = REAL // QN           # 3125 real rows per (core, quarter)
WIN = NC * QROWS             # 25600 <= 32767 (int16 gather indices)
NWIN = QN
RW = 128                 # bf16 elems per table row (256B)
BATCH_KMAX = int(_os.environ.get("KBM", "64"))  # max sum-of-K per gather call
CHUNK = 25               # combine blocks per chunk
ADST_GMAX = int(_os.environ.get("KADG", "64"))
# cap groups per batch: bounds the packed-partial-rows (pb) tile at
# NGMAX*256B/partition instead of BATCH_KMAX*256B, freeing SBUF for a
# deeper gather pool
NGMAX = int(_os.environ.get("KNG", "32"))


def _wrap_idx(flat):
    n = flat.shape[0]
    assert n % 16 == 0
    w16 = flat.reshape(n // 16, 16).T
    return np.tile(w16, (8, 1)).astype(np.int16)


# Pad slots must NOT all hit one ghost row: 17% of descriptors hammering a
# single 256B HBM row serializes one bank and slows the whole gather stream
# ~2.8x (5.3 vs 1.9 ns/desc measured). Spread over all NC*(QROWS-QREAL)
# ghost rows.
GHOST_ROWS = np.concatenate(
    [c * QROWS + QREAL + np.arange(QROWS - QREAL) for c in range(NC)])


def preprocess(edge_index):
    # Self-loops (PyG GATConv default) are NOT materialized as edges: the
    # dst node's own contribution exp(lrelu(asrc+adst)) * [h|1] is computed
    # in the combine phases from a sequential Rshard read — saves one
    # gather descriptor per node per layer plus K-padding.
    src = np.asarray(edge_index[0], np.int64)
    dst = np.asarray(edge_index[1], np.int64)

    core = dst // REAL
    rl = dst % REAL
    r_local = (rl // QREAL) * QROWS + rl % QREAL      # padded dst row
    src_c = src // REAL
    src_r = src % REAL
    w = src_r // QREAL                                 # window == quarter
    rel = src_c * QROWS + src_r % QREAL                # rank within window

    key = (core * NWIN + w) * SHARD + r_local
    deg = np.bincount(key, minlength=NC * NWIN * SHARD).reshape(
        NC, NWIN, SHARD)

    vlists = {}
    nnz = np.zeros((NC, NWIN), int)
    for c in range(NC):
        for wi in range(NWIN):
            d = deg[c, wi]
            rs = np.nonzero(d)[0]
            order = np.argsort(-d[rs], kind="stable")
            rs = rs[order]
            vlists[(c, wi)] = (rs, d[rs])
            nnz[c, wi] = len(rs)

    G_w = [max(1, int(np.ceil(nnz[:, wi].max() / 128))) for wi in range(NWIN)]
    K_w = []
    for wi in range(NWIN):
        ks = np.zeros(G_w[wi], int)
        for c in range(NC):
            degs = vlists[(c, wi)][1]
            for g in range(G_w[wi]):
                if g * 128 < len(degs):
                    ks[g] = max(ks[g], degs[g * 128])
        ks = np.maximum(ks, 1)
        K_w.append(ks)

    # K-uniform batches: every group in a batch is padded to the batch max K
    # so per-group DVE ops fuse into single per-batch ops.
    batches_w = []
    for wi in range(NWIN):
        batches = []
        g0 = 0
        while g0 < G_w[wi]:
            kb = int(K_w[wi][g0])          # groups sorted by K desc
            nb = max(1, min(BATCH_KMAX // kb, G_w[wi] - g0, NGMAX))
            g1 = g0 + nb
            K_w[wi][g0:g1] = kb            # pad K uniform within batch
            batches.append((g0, g1, int(kb * nb)))
            g0 = g1
        batches_w.append(batches)

    cumK_w = [np.concatenate([[0], np.cumsum(K_w[wi])]) for wi in range(NWIN)]

    sortpos = np.full((NC, NWIN, SHARD), -1, np.int64)
    for c in range(NC):
        for wi in range(NWIN):
            rs = vlists[(c, wi)][0]
            sortpos[c, wi, rs] = np.arange(len(rs))
    vpos = sortpos[core, w, r_local]
    order = np.argsort(key, kind="stable")
    k_in_row = np.empty(len(key), np.int64)
    sk_ = key[order]
    first = np.concatenate([[True], sk_[1:] != sk_[:-1]])
    starts = np.nonzero(first)[0]
    run_id = np.cumsum(first) - 1
    k_in_row[order] = np.arange(len(key)) - starts[run_id]

    gv = vpos // 128
    pv = vpos % 128

    idx_e_cores, idx_a_cores, idx_c_cores = [], [], []
    for c in range(NC):
        e_parts = []
        m_c = core == c
        for wi in range(NWIN):
            ncol = int(cumK_w[wi][-1])
            A = GHOST_ROWS[np.arange(128 * ncol) % len(GHOST_ROWS)].reshape(
                128, ncol)
            m = m_c & (w == wi)
            col = cumK_w[wi][gv[m]] + k_in_row[m]
            A[pv[m], col] = rel[m]
            for (g0, g1, _sk) in batches_w[wi]:
                c0, c1 = int(cumK_w[wi][g0]), int(cumK_w[wi][g1])
                e_parts.append(_wrap_idx(A[:, c0:c1].T.reshape(-1)))
        idx_e_cores.append(np.concatenate(e_parts, axis=1))

        a_parts = []
        for wi in range(NWIN):
            rs = vlists[(c, wi)][0]
            R_flat = np.zeros(G_w[wi] * 128, np.int64)
            R_flat[: len(rs)] = rs
            R = R_flat.reshape(G_w[wi], 128).T
            g0 = 0
            while g0 < G_w[wi]:
                g1 = min(g0 + ADST_GMAX, G_w[wi])
                a_parts.append(_wrap_idx(R[:, g0:g1].T.reshape(-1)))
                g0 = g1
        idx_a_cores.append(np.concatenate(a_parts, axis=1))

        c_parts = []
        for wi in range(NWIN):
            zr = 128 * G_w[wi]
            # spread no-virtual-row pads over 128 zero rows (same bank-
            # hammering fix as GHOST_ROWS)
            Cidx = zr + np.arange(SHARD, dtype=np.int64) % 128
            rs = vlists[(c, wi)][0]
            vp = np.arange(len(rs))
            Cidx[rs] = (vp % 128) * G_w[wi] + (vp // 128)
            Cm = Cidx.reshape(BLOCKS, 128).T
            for b0 in range(0, BLOCKS, CHUNK):
                b1 = min(b0 + CHUNK, BLOCKS)
                c_parts.append(_wrap_idx(Cm[:, b0:b1].T.reshape(-1)))
        idx_c_cores.append(np.concatenate(c_parts, axis=1))

    struct = dict(
        G_w=G_w, K_w=K_w, batches_w=batches_w, cumK_w=cumK_w,
        idx_e_w=idx_e_cores[0].shape[1], idx_a_w=idx_a_cores[0].shape[1],
        idx_c_w=idx_c_cores[0].shape[1],
    )
    return struct, idx_e_cores, idx_a_cores, idx_c_cores


# -----------------------------------------------------------------------
def build(struct):
    import os
    STAGE = int(os.environ.get("KSTAGE", "5"))
    KEDGE = int(os.environ.get("KEDGE", "3"))
    KREPS = int(os.environ.get("KREPS", "1"))
    KAG = int(os.environ.get("KAG", "1"))          # 0: fake AG with local copy
    # 2: skip AG entirely (timing-only; tables hold garbage)
    KADST = int(os.environ.get("KADST", "1"))      # 0: skip adst gathers
    # 1: AllGather only the used row columns (88 for L1, 42 for L2) via
    # strided APs — cuts AG wire bytes ~45%
    KAGW = int(os.environ.get("KAGW", "0"))
    # 0: Local AG outputs (fastest with pipelined quarter-AGs; >4 Shared
    # collectives also crash the runtime), 1: all Shared, 2: layer-1 Shared
    KSHARED = int(os.environ.get("KSHARED", "0"))
    KSP = bool(int(os.environ.get("KSP", "0")))    # single_packet for gathers
    # 1: leaky-relu on ACT engine (removes 2 DVE ops/batch, ~290 us; HW
    # Lrelu numerics are looser — rel err 1.4e-3 vs 4.8e-5, gate is 2e-2)
    KLR = int(os.environ.get("KLR", "1"))
    # 1: use Prelu instead of Lrelu — same math (x>0 ? x : alpha*x) but
    # Prelu shares the act-table set with Exp, killing ~126 table reloads
    KPRELU = int(os.environ.get("KPRELU", "1"))
    KEB = int(os.environ.get("KEB", "0"))          # 1: bf16 edge logits
    KCB = int(os.environ.get("KCB", "1"))          # deeper combine-gather
    # staging (cw tiles own pool): prefetch next chunk, -375 us
    KBUF = int(os.environ.get("KBUF", "0"))        # 1: trade a gather buf
    # for deeper compute pools (gp 3, ep/epb 3, epm 2); 0 wins now that
    # gathers are fast (ghost-spread fix): 3.60 vs 3.85 ms
    # gather-pool depth: deeper keeps more dma_gathers in flight (4->2.9ms,
    # 7->2.66ms); 7x2MB fits SBUF alongside the edge compute pools
    KGB = int(os.environ.get("KGB", "7"))
    import concourse.bacc as bacc
    import concourse.mybir as mybir
    import concourse.tile as tile
    from concourse.masks import make_identity

    F32 = mybir.dt.float32
    BF = mybir.dt.bfloat16
    I16 = mybir.dt.int16
    AX = mybir.AxisListType.X
    OP = mybir.AluOpType
    ACT = mybir.ActivationFunctionType

    G_w, K_w, batches_w = struct["G_w"], struct["K_w"], struct["batches_w"]
    cumK_w = struct["cumK_w"]
    KMAX = int(max(max(k) for k in K_w))
    FC = F_IN // 128

    nc = bacc.Bacc("TRN2", target_bir_lowering=False, debug=False,
                   num_devices=NC, num_swdge_queues=4)

    xt = nc.dram_tensor("xt", [F_IN, SHARD], F32, kind="ExternalInput").ap()
    w1 = nc.dram_tensor("w1", [F_IN, HD1], F32, kind="ExternalInput").ap()
    w2 = nc.dram_tensor("w2", [HD1, C], F32, kind="ExternalInput").ap()
    vec_in = {}
    for nm, width in [("atts1", HD1), ("attd1", HD1), ("b1", HD1),
                      ("atts2", C), ("attd2", C), ("b2", C)]:
        vec_in[nm] = nc.dram_tensor(nm, [1, width], F32,
                                    kind="ExternalInput").ap()
    idx_e = nc.dram_tensor("idx_e", [128, struct["idx_e_w"]], I16,
                           kind="ExternalInput").ap()
    idx_a = nc.dram_tensor("idx_a", [128, struct["idx_a_w"]], I16,
                           kind="ExternalInput").ap()
    idx_c = nc.dram_tensor("idx_c", [128, struct["idx_c_w"]], I16,
                           kind="ExternalInput").ap()
    out = nc.dram_tensor("out", [SHARD, C], F32, kind="ExternalOutput").ap()

    rg = [list(range(NC))]
    PT_rows = [128 * G_w[wi] + 128 for wi in range(NWIN)]
    PT_total = sum(PT_rows)
    PT_base = np.concatenate([[0], np.cumsum(PT_rows)]).astype(int)

    with tile.TileContext(nc) as tc:
        for _rep in range(KREPS):
            with (
                tc.tile_pool(name="dram", bufs=1, space="DRAM") as dpool,
                tc.tile_pool(name="setup", bufs=1) as sup,
                tc.tile_pool(name="psum0", bufs=2, space="PSUM") as psp,
            ):
                Rshard1 = dpool.tile([SHARD, RW], BF, tag="rs1")
                Rshard2 = dpool.tile([SHARD, RW], BF, tag="rs2")
                # KSHARED: 0 = all Local, 1 = all Shared, 2 = layer-1 only
                asp1 = "Shared" if KSHARED in (1, 2) else "Local"
                asp2 = "Shared" if KSHARED == 1 else "Local"
                Rw1 = [dpool.tile([WIN, RW], BF, tag=f"rf1{q}",
                                  addr_space=asp1, name=f"Rw1_{q}")
                       for q in range(QN)]
                Rw2 = [dpool.tile([WIN, RW], BF, tag=f"rf2{q}",
                                  addr_space=asp2, name=f"Rw2_{q}")
                       for q in range(QN)]
                AdstT = dpool.tile([SHARD, RW], BF, tag="adt")
                Ptab1 = dpool.tile([PT_total, RW], BF, tag="pt1")
                Ptab2 = dpool.tile([PT_total, RW], BF, tag="pt2")

                ident = sup.tile([128, 128], F32)
                make_identity(nc, ident[:])
                ones_row = sup.tile([1, 128], F32)
                nc.vector.memset(ones_row[:], 1.0)

                w1_t = sup.tile([128, FC * HD1], F32)
                nc.sync.dma_start(
                    w1_t[:].rearrange("p (c n) -> p c n", c=FC),
                    w1.rearrange("(c p) n -> p c n", p=128),
                )
                w2_t = sup.tile([128, C], F32)
                nc.sync.dma_start(w2_t[0:HD1, :], w2[:, :])
                nc.sync.dma_start(w2_t[HD1:2 * HD1, :], w2[:, :])

                reps = {}
                for nm in ["atts1", "attd1", "b1", "atts2", "attd2", "b2"]:
                    width = HD1 if nm in ("atts1", "attd1", "b1") else C
                    v = sup.tile([1, width], F32, tag=f"v_{nm}")
                    nc.sync.dma_start(v[:], vec_in[nm][:, :])
                    ps = psp.tile([128, width], F32, tag="rep_ps")
                    nc.tensor.matmul(out=ps[:], lhsT=ones_row[:], rhs=v[:],
                                     start=True, stop=True)
                    r_ = sup.tile([128, width], F32, tag=f"rep_{nm}")
                    nc.vector.tensor_copy(r_[:], ps[:])
                    reps[nm] = r_

                ghost1 = sup.tile([128, 88], BF)
                nc.vector.memset(ghost1[:], 0.0)
                nc.vector.memset(ghost1[:, 72:80], -100.0)
                zrow = sup.tile([128, RW], BF)
                nc.vector.memset(zrow[:], 0.0)
                for wi in range(NWIN):
                    zr = int(PT_base[wi]) + 128 * G_w[wi]
                    nc.sync.dma_start(Ptab1[:][zr:zr + 128, :], zrow[:])
                    nc.sync.dma_start(Ptab2[:][zr:zr + 128, :], zrow[:])

                # ---------------- dense layer 1 ----------------
                with (
                    tc.tile_pool(name="d1", bufs=3) as dp,
                    tc.tile_pool(name="d1p", bufs=2, space="PSUM") as dpp,
                ):
                    for t in range(BLOCKS):
                        xtile = dp.tile([128, FC * 128], F32, tag="x")
                        nc.sync.dma_start(
                            xtile[:].rearrange("p (c n) -> p c n", c=FC),
                            xt.rearrange("(c p) n -> p c n", p=128)[
                                :, :, t * 128:(t + 1) * 128],
                        )
                        hps = dpp.tile([128, HD1], F32, tag="h")
                        for cc in range(FC):
                            nc.tensor.matmul(
                                out=hps[:],
                                lhsT=xtile[:].rearrange(
                                    "p (c n) -> p c n", c=FC)[:, cc, :],
                                rhs=w1_t[:].rearrange(
                                    "p (c n) -> p c n", c=FC)[:, cc, :],
                                start=(cc == 0), stop=(cc == FC - 1),
                            )
                        # row layout v2: head h at [9h:9h+8], 1.0 at 9h+8
                        # (embedded softmax-denominator column), asrc@72,
                        # adst@80; [88:] never read.
                        row = dp.tile([128, RW], BF, tag="row")
                        rowh = row[:][:, 0:72].rearrange(
                            "p (h x) -> p h x", x=9)
                        nc.vector.tensor_copy(
                            rowh[:, :, 0:8],
                            hps[:].rearrange("p (h d) -> p h d", h=H1))
                        nc.vector.memset(rowh[:, :, 8:9], 1.0)
                        asrc_f = dp.tile([128, H1], F32, tag="asrcf")
                        adst_f = dp.tile([128, H1], F32, tag="adstf")
                        tmp = dp.tile([128, HD1], F32, tag="tmp")
                        for nm, dst_ap in (("atts1", asrc_f), ("attd1", adst_f)):
                            nc.vector.tensor_tensor(
                                out=tmp[:], in0=hps[:], in1=reps[nm][:],
                                op=OP.mult)
                            nc.vector.tensor_reduce(
                                out=dst_ap[:],
                                in_=tmp[:].rearrange("p (h d) -> p h d", h=H1),
                                axis=AX, op=OP.add)
                        nc.vector.tensor_copy(row[:, 72:80], asrc_f[:])
                        nc.vector.tensor_copy(row[:, 80:88], adst_f[:])
                        nc.sync.dma_start(Rshard1[t * 128:(t + 1) * 128, :],
                                          row[:])
                        arow = dp.tile([128, 9], BF, tag="arow")
                        nc.vector.tensor_copy(arow[:, 0:8], adst_f[:])
                        nc.vector.memset(arow[:, 8:9], 0.0)
                        nc.sync.dma_start(AdstT[t * 128:(t + 1) * 128, 0:9],
                                          arow[:])

                        # quarter q done after its 25 blocks: write its ghost
                        # rows and launch its AllGather so transfer overlaps
                        # the remaining dense blocks and later edge windows
                        if (t + 1) % (QROWS // 128) == 0:
                            q = t // (QROWS // 128)
                            g0r = q * QROWS + QREAL
                            nc.sync.dma_start(
                                Rshard1[g0r:(q + 1) * QROWS, 0:88],
                                ghost1[: QROWS - QREAL, :])
                            if KAG == 1:
                                w1hi = 88 if KAGW else RW
                                nc.gpsimd.collective_compute(
                                    "AllGather", OP.bypass, replica_groups=rg,
                                    ins=[Rshard1[:][q * QROWS:(q + 1) * QROWS,
                                                    0:w1hi].opt()],
                                    outs=[Rw1[q][:][:, 0:w1hi].opt()])
                            elif KAG == 2:
                                nc.sync.dma_start(
                                    Rw1[q][:][0:128, :],
                                    Rshard1[q * QROWS:q * QROWS + 128, :])
                            else:
                                for c in range(NC):
                                    nc.sync.dma_start(
                                        Rw1[q][:][c * QROWS:
                                                  (c + 1) * QROWS, :],
                                        Rshard1[q * QROWS:(q + 1) * QROWS, :])

                # ---------------- edge phase ----------------
                qn = [0, None]

                def edge_phase(Rws, Ptab, layer):
                    e_col = 0
                    a_col = 0
                    # v2 rows: layer 1 packs head h at [9h:9h+9] (8 feats +
                    # const 1.0 denominator column), asrc@72, adst@0 in AdstT;
                    # layer 2: h2[0:40], 1.0@40, asrc@41, adst@8 in AdstT.
                    if layer == 1:
                        Hh, Xw, alo, dlo = H1, D1 + 1, 72, 0
                    else:
                        Hh, Xw, alo, dlo = 1, C + 1, 41, 8
                    width = Hh * Xw
                    for wi in range(NWIN):
                        Rwin = Rws[wi]
                        Gn = G_w[wi]
                        ecols_w = 8 * sum(sk for (_g0, _g1, sk) in batches_w[wi])
                        acols_w = Gn * 8
                        with (
                            tc.tile_pool(name=f"ad{layer}{wi}", bufs=1) as apool,
                            tc.tile_pool(name=f"eg{layer}{wi}",
                                         bufs=KGB or (4 - KBUF)) as gp,
                            tc.tile_pool(name=f"ep{layer}{wi}",
                                         bufs=2 + KBUF) as ep,
                            tc.tile_pool(name=f"em{layer}{wi}",
                                         bufs=1 + KBUF) as epm,
                            tc.tile_pool(name=f"eb{layer}{wi}",
                                         bufs=2 + KBUF) as epb,
                            tc.tile_pool(name=f"ix{layer}{wi}", bufs=1) as ixp,
                        ):
                            ixw = ixp.tile([128, ecols_w], I16, tag="ixw")
                            nc.sync.dma_start(ixw[:], idx_e[:, e_col: e_col + ecols_w])
                            ixaw = ixp.tile([128, acols_w], I16, tag="ixaw")
                            nc.sync.dma_start(ixaw[:], idx_a[:, a_col: a_col + acols_w])
                            ecol_loc = 0
                            acol_loc = 0
                            adstG = apool.tile([128, Gn * RW], BF, tag="adstG")
                            adstG_v = adstG[:].rearrange("p (g e) -> p g e", e=RW)
                            if not KADST:
                                nc.vector.memset(adstG[:], 0.0)
                            g0 = 0
                            while g0 < Gn:
                                g1 = min(g0 + ADST_GMAX, Gn)
                                nidx = (g1 - g0) * 128
                                if KADST:
                                    _gi = nc.gpsimd.dma_gather(
                                        out_ap=adstG_v[:, g0:g1, :],
                                        in_ap=AdstT[:, :],
                                        idxs_ap=ixaw[:, acol_loc: acol_loc + nidx // 16],
                                        num_idxs=nidx, num_idxs_reg=nidx,
                                        elem_size=RW, single_packet=KSP,
                                        queue_num=qn[0] % 4)
                                    if qn[1] is not None:
                                        tile.add_dep_helper(_gi.ins, qn[1].ins, sync=False,
                                                            reason="swdge order")
                                    qn[1] = _gi
                                    qn[0] += 1
                                a_col += nidx // 16
                                acol_loc += nidx // 16
                                g0 = g1

                            for bidx, (g0, g1, sk) in enumerate(batches_w[wi]):
                                Kb = int(K_w[wi][g0])
                                ng = g1 - g0
                                nidx = 128 * sk
                                ixe = ixw[:, ecol_loc: ecol_loc + nidx // 16]
                                e_col += nidx // 16
                                ecol_loc += nidx // 16
                                G = gp.tile([128, BATCH_KMAX * RW], BF, tag="G")
                                Gv = G[:].rearrange("p (k e) -> p k e", e=RW)
                                Gg = G[:][:, 0:sk * RW].rearrange(
                                    "p (g k e) -> p g k e", g=ng, k=Kb)
                                _gi = nc.gpsimd.dma_gather(
                                    out_ap=Gv[:, 0:sk, :],
                                    in_ap=Rwin[:, :],
                                    idxs_ap=ixe,
                                    num_idxs=nidx, num_idxs_reg=nidx,
                                    elem_size=RW, single_packet=KSP,
                                    queue_num=qn[0] % 4)
                                if qn[1] is not None:
                                    tile.add_dep_helper(_gi.ins, qn[1].ins, sync=False,
                                                        reason="swdge order")
                                qn[1] = _gi
                                qn[0] += 1

                                if KEDGE < 2:
                                    continue
                                EDT = BF if KEB else F32
                                eT = ep.tile([128, BATCH_KMAX * Hh], EDT,
                                             tag="eT")
                                eV = eT[:][:, 0:sk * Hh].rearrange(
                                    "p (g k h) -> p g k h", g=ng, k=Kb)
                                pT = ep.tile([128, BATCH_KMAX * Hh], BF, tag="pT")
                                pb = epb.tile([128, NGMAX * RW], BF, tag="pb")
                                pbV = pb[:].rearrange("p (g e) -> p g e", e=RW)

                                # e = a_src[src] + a_dst (one op per batch)
                                nc.vector.tensor_tensor(
                                    out=eV[:, :, :, :],
                                    in0=Gg[:, :, :, alo:alo + Hh],
                                    in1=adstG_v[:, g0:g1, dlo:dlo + Hh]
                                        .unsqueeze(2)
                                        .to_broadcast([128, ng, Kb, Hh]),
                                    op=OP.add)
                                # leaky relu + exp
                                if KLR:
                                    eL = ep.tile([128, BATCH_KMAX * Hh], EDT,
                                                 tag="eL")
                                    nc.scalar.activation(
                                        eL[:, : sk * Hh], eT[:, : sk * Hh],
                                        ACT.Prelu if KPRELU else ACT.Lrelu,
                                        alpha=NEG_SLOPE)
                                    nc.scalar.activation(
                                        pT[:, : sk * Hh], eL[:, : sk * Hh],
                                        ACT.Exp)
                                else:
                                    ee = ep.tile([128, BATCH_KMAX * Hh], EDT,
                                                 tag="ee")
                                    nc.vector.tensor_scalar_mul(
                                        ee[:, : sk * Hh], eT[:, : sk * Hh],
                                        NEG_SLOPE)
                                    nc.vector.tensor_tensor(
                                        out=eT[:, : sk * Hh],
                                        in0=eT[:, : sk * Hh],
                                        in1=ee[:, : sk * Hh], op=OP.max)
                                    nc.scalar.activation(
                                        pT[:, : sk * Hh], eT[:, : sk * Hh],
                                        ACT.Exp)

                                # msg = p * [h | 1] (bf16; denominator rides in
                                # the const-1 column, so no separate reduce)
                                msg = epm.tile([128, BATCH_KMAX * width], BF,
                                               tag="msg")
                                msgV = msg[:][:, 0:sk * width].rearrange(
                                    "p (g k f) -> p g k f", g=ng, k=Kb)
                                nc.vector.tensor_tensor(
                                    out=msg[:][:, 0:sk * width].rearrange(
                                        "p (k h x) -> p k h x", k=sk, h=Hh),
                                    in0=Gv[:, 0:sk, 0:width].rearrange(
                                        "p k (h x) -> p k h x", h=Hh),
                                    in1=pT[:][:, 0:sk * Hh].rearrange(
                                        "p (k h) -> p k h", h=Hh)
                                        .unsqueeze(3)
                                        .to_broadcast([128, sk, Hh, Xw]),
                                    op=OP.mult)
                                # tree-reduce over k (uniform Kb, bf16)
                                kk = Kb
                                while kk > 1:
                                    half = kk // 2
                                    nc.vector.tensor_tensor(
                                        out=msgV[:, :, 0:half, :],
                                        in0=msgV[:, :, 0:half, :],
                                        in1=msgV[:, :, half:2 * half, :],
                                        op=OP.add)
                                    if kk % 2 == 1:
                                        nc.vector.tensor_tensor(
                                            out=msgV[:, :, 0:1, :],
                                            in0=msgV[:, :, 0:1, :],
                                            in1=msgV[:, :, kk - 1:kk, :],
                                            op=OP.add)
                                    kk = half
                                # pack partial rows ([width:] stays garbage —
                                # never read by the combine phase)
                                nc.vector.tensor_copy(
                                    pbV[:, 0:ng, 0:width],
                                    msgV[:, :, 0, :])
                                if KEDGE >= 3:
                                    nc.sync.dma_start(
                                        Ptab[:][int(PT_base[wi]):
                                                int(PT_base[wi]) + 128 * Gn, :]
                                        .rearrange("(p g) e -> p g e", p=128)
                                        [:, g0:g1, :],
                                        pbV[:, 0:ng, :])

                if STAGE >= 2:
                    edge_phase(Rw1, Ptab1, 1)

                # ------------- combine helpers -------------
                def combine_chunks(Ptab, body, cp, cxp, tagp, post=None,
                                   gcp=None):
                    ixcw = cxp.tile([128, NWIN * BLOCKS * 8], I16, tag="ixcw")
                    nc.sync.dma_start(ixcw[:], idx_c[:, :])
                    for b0 in range(0, BLOCKS, CHUNK):
                        b1 = min(b0 + CHUNK, BLOCKS)
                        nb = b1 - b0
                        CWs = []
                        for wi in range(NWIN):
                            nidx = nb * 128
                            off = (wi * BLOCKS + b0) * 128 // 16
                            ixc = ixcw[:, off: off + nidx // 16]
                            CW = (gcp or cp).tile(
                                [128, CHUNK * RW], BF,
                                tag=f"cw{tagp}{wi}")
                            _gi = nc.gpsimd.dma_gather(
                                out_ap=CW[:].rearrange(
                                    "p (b e) -> p b e", e=RW)[:, 0:nb, :],
                                in_ap=Ptab[:][int(PT_base[wi]):
                                              int(PT_base[wi]) + PT_rows[wi], :],
                                idxs_ap=ixc,
                                num_idxs=nidx, num_idxs_reg=nidx,
                                elem_size=RW, single_packet=KSP,
                                queue_num=qn[0] % 4)
                            if qn[1] is not None:
                                tile.add_dep_helper(_gi.ins, qn[1].ins, sync=False,
                                                    reason="swdge order")
                            qn[1] = _gi
                            qn[0] += 1
                            CWs.append(CW[:].rearrange("p (b e) -> p b e", e=RW))
                        body(b0, b1, CWs)
                        if post is not None:
                            post(b0 // CHUNK)

                def add4(cp, CWs, nb, lo, hi, ftag):
                    width = hi - lo
                    acc = cp.tile([128, CHUNK * width], F32, tag=f"acc{ftag}")
                    t0 = cp.tile([128, CHUNK * width], F32, tag=f"t0{ftag}")
                    accV = acc[:].rearrange("p (b f) -> p b f", f=width)
                    t0V = t0[:].rearrange("p (b f) -> p b f", f=width)
                    nc.vector.tensor_tensor(
                        out=accV[:, 0:nb], in0=CWs[0][:, 0:nb, lo:hi],
                        in1=CWs[1][:, 0:nb, lo:hi], op=OP.add)
                    if NWIN > 2:
                        nc.vector.tensor_tensor(
                            out=t0V[:, 0:nb], in0=CWs[2][:, 0:nb, lo:hi],
                            in1=CWs[3][:, 0:nb, lo:hi], op=OP.add)
                        nc.vector.tensor_tensor(
                            out=accV[:, 0:nb], in0=accV[:, 0:nb],
                            in1=t0V[:, 0:nb], op=OP.add)
                    return accV

                def add4_f32(cp, CWs, nb, lo, nf, ftag):
                    acc = cp.tile([128, CHUNK * nf], F32, tag=f"acs{ftag}")
                    t0 = cp.tile([128, CHUNK * nf], F32, tag=f"ts{ftag}")
                    accV = acc[:].rearrange("p (b f) -> p b f", f=nf)
                    t0V = t0[:].rearrange("p (b f) -> p b f", f=nf)
                    nc.vector.tensor_tensor(
                        out=accV[:, 0:nb],
                        in0=CWs[0][:, 0:nb, lo:lo + 2 * nf].bitcast(F32),
                        in1=CWs[1][:, 0:nb, lo:lo + 2 * nf].bitcast(F32),
                        op=OP.add)
                    if NWIN > 2:
                        nc.vector.tensor_tensor(
                            out=t0V[:, 0:nb],
                            in0=CWs[2][:, 0:nb, lo:lo + 2 * nf].bitcast(F32),
                            in1=CWs[3][:, 0:nb, lo:lo + 2 * nf].bitcast(F32),
                            op=OP.add)
                        nc.vector.tensor_tensor(
                            out=accV[:, 0:nb], in0=accV[:, 0:nb],
                            in1=t0V[:, 0:nb], op=OP.add)
                    return accV

                # ---------------- combine L1 + dense layer 2 ----------------
                if STAGE >= 3:
                  with (
                      tc.tile_pool(name="c1", bufs=2) as cp,
                      tc.tile_pool(name="c1g", bufs=2 + KCB) as gcp1,
                      tc.tile_pool(name="c1x", bufs=2) as cxp,
                      tc.tile_pool(name="c1s", bufs=1) as slp,
                      tc.tile_pool(name="c1p", bufs=2, space="PSUM") as cpp,
                  ):
                      def c1_body(b0, b1, CWs):
                          nb = b1 - b0
                          UcV = add4(cp, CWs, nb, 0, 72, "u1")
                          # self-loop term: U += exp(lrelu(asrc+adst)) * [h|1]
                          r1c = slp.tile([128, CHUNK * RW], BF, tag="r1c")
                          r1V = r1c[:].rearrange("p (b e) -> p b e", e=RW)
                          nc.sync.dma_start(
                              r1V[:, 0:nb, :],
                              Rshard1[b0 * 128:b1 * 128, :].rearrange(
                                  "(b p) e -> p b e", p=128))
                          es = slp.tile([128, CHUNK * H1], F32, tag="es1")
                          nc.vector.tensor_tensor(
                              out=es[:][:, 0:nb * H1].rearrange(
                                  "p (b h) -> p b h", b=nb),
                              in0=r1V[:, 0:nb, 72:80],
                              in1=r1V[:, 0:nb, 80:88], op=OP.add)
                          el = slp.tile([128, CHUNK * H1], F32, tag="el1")
                          nc.scalar.activation(
                              el[:, 0:nb * H1], es[:, 0:nb * H1],
                              ACT.Prelu if KPRELU else ACT.Lrelu,
                              alpha=NEG_SLOPE)
                          nc.scalar.activation(es[:, 0:nb * H1],
                                               el[:, 0:nb * H1], ACT.Exp)
                          sm = slp.tile([128, CHUNK * 72], F32, tag="sm1")
                          smV = sm[:][:, 0:nb * 72].rearrange(
                              "p (b h x) -> p b h x", b=nb, h=H1)
                          nc.vector.tensor_tensor(
                              out=smV,
                              in0=r1V[:, 0:nb, 0:72].rearrange(
                                  "p b (h x) -> p b h x", x=9),
                              in1=es[:][:, 0:nb * H1].rearrange(
                                  "p (b h) -> p b h", b=nb).unsqueeze(3)
                                  .to_broadcast([128, nb, H1, 9]),
                              op=OP.mult)
                          nc.vector.tensor_tensor(
                              out=UcV, in0=UcV,
                              in1=sm[:][:, 0:nb * 72].rearrange(
                                  "p (b f) -> p b f", b=nb),
                              op=OP.add)
                          Ux = UcV.rearrange("p b (h x) -> p b h x", x=9)
                          rinv = cp.tile([128, CHUNK * H1], F32, tag="rinv")
                          nc.vector.tensor_scalar(
                              out=rinv[:][:, 0:nb * H1].rearrange(
                                  "p (b h) -> p b h", b=nb),
                              in0=Ux[:, :, :, 8], scalar1=EPS,
                              scalar2=None, op0=OP.add)
                          nc.vector.reciprocal(rinv[:, 0:nb * H1],
                                               rinv[:, 0:nb * H1])
                          o1c = cp.tile([128, CHUNK * HD1], F32, tag="o1c")
                          nc.vector.tensor_tensor(
                              out=o1c[:][:, 0:nb * HD1].rearrange(
                                  "p (b h d) -> p b h d", b=nb, h=H1),
                              in0=Ux[:, :, :, 0:D1],
                              in1=rinv[:][:, 0:nb * H1].rearrange(
                                  "p (b h) -> p b h", b=nb).unsqueeze(3)
                                  .to_broadcast([128, nb, H1, D1]),
                              op=OP.mult)
                          nc.vector.tensor_tensor(
                              out=o1c[:][:, 0:nb * HD1].rearrange(
                                  "p (b f) -> p b f", b=nb),
                              in0=o1c[:][:, 0:nb * HD1].rearrange(
                                  "p (b f) -> p b f", b=nb),
                              in1=reps["b1"][:].unsqueeze(1)
                                  .to_broadcast([128, nb, HD1]),
                              op=OP.add)
                          of = o1c[:, 0:nb * HD1]
                          mn = cp.tile([128, CHUNK * HD1], F32, tag="mn")
                          nc.vector.tensor_scalar(
                              out=mn[:, 0:nb * HD1], in0=of, scalar1=0.0,
                              scalar2=None, op0=OP.min)
                          ex = cp.tile([128, CHUNK * HD1], F32, tag="ex")
                          nc.scalar.activation(ex[:, 0:nb * HD1], mn[:, 0:nb * HD1],
                                               ACT.Exp)
                          nc.vector.tensor_scalar(
                              out=of, in0=of, scalar1=0.0, scalar2=None, op0=OP.max)
                          nc.vector.tensor_tensor(
                              out=of, in0=of, in1=ex[:, 0:nb * HD1], op=OP.add)
                          nc.vector.tensor_scalar(
                              out=of, in0=of, scalar1=-1.0, scalar2=None, op0=OP.add)
                          # h2 = elu @ W2 : transpose 2 blocks at a time
                          h2c = cp.tile([128, CHUNK * C], F32, tag="h2c")
                          h2cV = h2c[:][:, 0:nb * C].rearrange(
                              "p (b f) -> p b f", b=nb)
                          for bp in range(0, nb, 2):
                              npair = min(2, nb - bp)
                              tp = cpp.tile([128, 128], F32, tag="tp")
                              nc.tensor.transpose(
                                  out=tp[0:npair * HD1, :],
                                  in_=o1c[:, bp * HD1:(bp + npair) * HD1],
                                  identity=ident[:])
                              eT_ = cp.tile([128, 128], F32, tag="eT2")
                              nc.vector.tensor_copy(eT_[0:npair * HD1, :],
                                                    tp[0:npair * HD1, :])
                              for j in range(npair):
                                  h2p = cpp.tile([128, C], F32, tag="h2p")
                                  nc.tensor.matmul(
                                      out=h2p[:],
                                      lhsT=eT_[j * HD1:(j + 1) * HD1, :],
                                      rhs=w2_t[j * HD1:(j + 1) * HD1, :],
                                      start=True, stop=True)
                                  nc.vector.tensor_copy(
                                      h2c[:, (bp + j) * C:(bp + j + 1) * C],
                                      h2p[:])
                          # attention logits for layer 2 (chunk-wide)
                          tmp2 = cp.tile([128, CHUNK * C], F32, tag="tmp2")
                          a2s = cp.tile([128, CHUNK], F32, tag="a2s")
                          a2d = cp.tile([128, CHUNK], F32, tag="a2d")
                          for nm, dst in (("atts2", a2s), ("attd2", a2d)):
                              nc.vector.tensor_tensor(
                                  out=tmp2[:][:, 0:nb * C].rearrange(
                                      "p (b f) -> p b f", b=nb),
                                  in0=h2cV,
                                  in1=reps[nm][:].unsqueeze(1)
                                      .to_broadcast([128, nb, C]),
                                  op=OP.mult)
                              nc.vector.tensor_reduce(
                                  out=dst[:, 0:nb],
                                  in_=tmp2[:][:, 0:nb * C].rearrange(
                                      "p (b f) -> p b f", b=nb),
                                  axis=AX, op=OP.add)
                          row2c = cp.tile([128, CHUNK * RW], BF, tag="row2c")
                          r2V = row2c[:].rearrange("p (b e) -> p b e", e=RW)
                          nc.vector.tensor_copy(r2V[:, 0:nb, 0:C], h2cV)
                          nc.vector.memset(r2V[:, 0:nb, 40:41], 1.0)
                          nc.vector.tensor_copy(
                              r2V[:, 0:nb, 41:42],
                              a2s[:][:, 0:nb].unsqueeze(2))
                          # adst2 rides at col 42 for the combine-2 self term
                          nc.vector.tensor_copy(
                              r2V[:, 0:nb, 42:43],
                              a2d[:][:, 0:nb].unsqueeze(2))
                          nc.sync.dma_start(
                              Rshard2[b0 * 128:b1 * 128, :].rearrange(
                                  "(b p) e -> p b e", p=128),
                              r2V[:, 0:nb, :])
                          adr = cp.tile([128, CHUNK], BF, tag="adr")
                          nc.vector.tensor_copy(adr[:, 0:nb], a2d[:, 0:nb])
                          nc.sync.dma_start(
                              AdstT[b0 * 128:b1 * 128, 8:9].rearrange(
                                  "(b p) e -> p b e", p=128),
                              adr[:][:, 0:nb].unsqueeze(2))

                      gz = cp.tile([128, 43], BF, tag="gz")
                      nc.vector.memset(gz[:, 0:41], 0.0)
                      nc.vector.memset(gz[:, 41:42], -100.0)
                      nc.vector.memset(gz[:, 42:43], 0.0)

                      # one combine chunk == one quarter: ghost-fill the
                      # quarter and launch its layer-2 AllGather immediately,
                      # overlapping later chunks and edge-2 windows
                      def c1_post(q):
                          if STAGE < 4:
                              return
                          g0r = q * QROWS + QREAL
                          nc.sync.dma_start(
                              Rshard2[g0r:(q + 1) * QROWS, 0:43],
                              gz[: QROWS - QREAL, :])
                          if KAG == 1:
                              w2hi = 42 if KAGW else RW
                              nc.gpsimd.collective_compute(
                                  "AllGather", OP.bypass, replica_groups=rg,
                                  ins=[Rshard2[:][q * QROWS:(q + 1) * QROWS,
                                                  0:w2hi].opt()],
                                  outs=[Rw2[q][:][:, 0:w2hi].opt()])
                          elif KAG == 2:
                              nc.sync.dma_start(
                                  Rw2[q][:][0:128, :],
                                  Rshard2[q * QROWS:q * QROWS + 128, :])
                          else:
                              for c in range(NC):
                                  nc.sync.dma_start(
                                      Rw2[q][:][c * QROWS:(c + 1) * QROWS, :],
                                      Rshard2[q * QROWS:(q + 1) * QROWS, :])

                      combine_chunks(Ptab1, c1_body, cp, cxp, "a", post=c1_post,
                                     gcp=gcp1)

                if STAGE >= 4:
                    edge_phase(Rw2, Ptab2, 2)

                # ---------------- combine L2 + log_softmax ----------------
                if STAGE >= 5:
                  with (
                      tc.tile_pool(name="c2", bufs=2) as cp2,
                      tc.tile_pool(name="c2g", bufs=2 + KCB) as gcp2,
                      tc.tile_pool(name="c2x", bufs=2) as cxp2,
                      tc.tile_pool(name="c2s", bufs=1) as slp2,
                  ):
                      def c2_body(b0, b1, CWs):
                          nb = b1 - b0
                          UcV = add4(cp2, CWs, nb, 0, C + 1, "u2")
                          # self-loop term: U += exp(lrelu(asrc2+adst2))*[h2|1]
                          r2c = slp2.tile([128, CHUNK * RW], BF, tag="r2c")
                          r2V = r2c[:].rearrange("p (b e) -> p b e", e=RW)
                          nc.sync.dma_start(
                              r2V[:, 0:nb, :],
                              Rshard2[b0 * 128:b1 * 128, :].rearrange(
                                  "(b p) e -> p b e", p=128))
                          es2 = slp2.tile([128, CHUNK], F32, tag="es2")
                          nc.vector.tensor_tensor(
                              out=es2[:][:, 0:nb].unsqueeze(2),
                              in0=r2V[:, 0:nb, 41:42],
                              in1=r2V[:, 0:nb, 42:43], op=OP.add)
                          el2 = slp2.tile([128, CHUNK], F32, tag="el2")
                          nc.scalar.activation(
                              el2[:, 0:nb], es2[:, 0:nb],
                              ACT.Prelu if KPRELU else ACT.Lrelu,
                              alpha=NEG_SLOPE)
                          nc.scalar.activation(es2[:, 0:nb], el2[:, 0:nb],
                                               ACT.Exp)
                          sm2 = slp2.tile([128, CHUNK * (C + 1)], F32,
                                          tag="sm2")
                          nc.vector.tensor_tensor(
                              out=sm2[:][:, 0:nb * (C + 1)].rearrange(
                                  "p (b f) -> p b f", b=nb),
                              in0=r2V[:, 0:nb, 0:C + 1],
                              in1=es2[:][:, 0:nb].unsqueeze(2)
                                  .to_broadcast([128, nb, C + 1]),
                              op=OP.mult)
                          nc.vector.tensor_tensor(
                              out=UcV, in0=UcV,
                              in1=sm2[:][:, 0:nb * (C + 1)].rearrange(
                                  "p (b f) -> p b f", b=nb),
                              op=OP.add)
                          rinv = cp2.tile([128, CHUNK], F32, tag="rinv2")
                          nc.vector.tensor_scalar(
                              out=rinv[:, 0:nb],
                              in0=UcV[:, :, C],
                              scalar1=EPS, scalar2=None, op0=OP.add)
                          nc.vector.reciprocal(rinv[:, 0:nb], rinv[:, 0:nb])
                          o2c = cp2.tile([128, CHUNK * C], F32, tag="o2c")
                          o2V = o2c[:][:, 0:nb * C].rearrange(
                              "p (b f) -> p b f", b=nb)
                          nc.vector.tensor_tensor(
                              out=o2V, in0=UcV[:, :, 0:C],
                              in1=rinv[:][:, 0:nb].unsqueeze(2)
                                  .to_broadcast([128, nb, C]),
                              op=OP.mult)
                          nc.vector.tensor_tensor(
                              out=o2V, in0=o2V,
                              in1=reps["b2"][:].unsqueeze(1)
                                  .to_broadcast([128, nb, C]),
                              op=OP.add)
                          mx = cp2.tile([128, CHUNK], F32, tag="mx")
                          nc.vector.tensor_reduce(
                              out=mx[:, 0:nb], in_=o2V, axis=AX, op=OP.max)
                          nc.vector.tensor_tensor(
                              out=o2V, in0=o2V,
                              in1=mx[:][:, 0:nb].unsqueeze(2)
                                  .to_broadcast([128, nb, C]),
                              op=OP.subtract)
                          ex2 = cp2.tile([128, CHUNK * C], F32, tag="ex2")
                          nc.scalar.activation(ex2[:, 0:nb * C], o2c[:, 0:nb * C],
                                               ACT.Exp)
                          ss = cp2.tile([128, CHUNK], F32, tag="ss")
                          nc.vector.tensor_reduce(
                              out=ss[:, 0:nb],
                              in_=ex2[:][:, 0:nb * C].rearrange(
                                  "p (b f) -> p b f", b=nb),
                              axis=AX, op=OP.add)
                          nc.scalar.activation(ss[:, 0:nb], ss[:, 0:nb], ACT.Ln)
                          nc.vector.tensor_tensor(
                              out=o2V, in0=o2V,
                              in1=ss[:][:, 0:nb].unsqueeze(2)
                                  .to_broadcast([128, nb, C]),
                              op=OP.subtract)
                          nc.sync.dma_start(
                              out[b0 * 128:b1 * 128, :].rearrange(
                                  "(b p) c -> p b c", p=128),
                              o2V)

                      combine_chunks(Ptab2, c2_body, cp2, cxp2, "b", gcp=gcp2)

    nc.compile()
    return nc


_CACHE = {}


def _in_maps(inputs, idx_e, idx_a, idx_c):
    x = np.asarray(inputs["x"], np.float32)
    maps = []
    for c in range(NC):
        xs = np.zeros((F_IN, SHARD), np.float32)
        for q in range(QN):
            xs[:, q * QROWS:q * QROWS + QREAL] = \
                x[c * REAL + q * QREAL:c * REAL + (q + 1) * QREAL].T
        maps.append({
            "xt": xs,
            "w1": np.asarray(inputs["W1"], np.float32),
            "w2": np.asarray(inputs["W2"], np.float32),
            "atts1": np.asarray(inputs["att_src1"], np.float32).reshape(1, HD1),
            "attd1": np.asarray(inputs["att_dst1"], np.float32).reshape(1, HD1),
            "b1": np.asarray(inputs["b1"], np.float32).reshape(1, HD1),
            "atts2": np.asarray(inputs["att_src2"], np.float32).reshape(1, C),
            "attd2": np.asarray(inputs["att_dst2"], np.float32).reshape(1, C),
            "b2": np.asarray(inputs["b2"], np.float32).reshape(1, C),
            "idx_e": idx_e[c], "idx_a": idx_a[c], "idx_c": idx_c[c],
        })
    return maps


def kernel(**inputs):
    from concourse import bass_utils

    struct, idx_e, idx_a, idx_c = preprocess(inputs["edge_index"])
    key = (struct["idx_e_w"], struct["idx_a_w"], struct["idx_c_w"],
           tuple(struct["G_w"]))
    if key not in _CACHE:
        _CACHE[key] = build(struct)
    nc = _CACHE[key]

    maps = _in_maps(inputs, idx_e, idx_a, idx_c)
    res = bass_utils.run_bass_kernel_spmd(nc, maps, core_ids=list(range(NC)))
    out = np.concatenate(
        [res.results[c]["out"].reshape(QN, QROWS, C)[:, :QREAL].reshape(-1, C)
         for c in range(NC)], axis=0)
    return out.astype(np.float32)


if __name__ == "__main__":
    rng = np.random.default_rng(0)
    ei = np.stack([rng.integers(0, N, E), rng.integers(0, N, E)])
    struct, *_ = preprocess(ei.astype(np.int64))
    print("G_w:", struct["G_w"])
    print("K sums:", [int(k.sum()) for k in struct["K_w"]])
    print("widths:", struct["idx_e_w"], struct["idx_a_w"], struct["idx_c_w"])

